# revision 1
# baseline (speedup 1.0000x reference)
"""Causal self-attention on 8 TRN2 NeuronCores (Bass/Tile, SPMD).

Problem: B=4, T=2048, C=1024, H=16, D=64, fp32 in/out.

Sharding: core i = (batch b=i//2, parity p=i%2). Each core computes ALL 16
heads for its interleaved quarter of query positions: 256-wide q-chunks
{0,3,4,7} (parity 0) or {1,2,5,6} (parity 1) of batch b. Slot-sorted by
causal prefix length, both parities' slots pad to extents {4,8,12,16}
t'-tiles -> every core runs the IDENTICAL instruction stream (SPMD), with
causality/padding handled by host-supplied mask data. K/V are computed for
the full sequence on both cores of a batch (cheap duplication beats any
collective here). No inter-core communication at all.

Per-core pipeline:
  1. K^T[d,t], Q^T[d,t_own] (d on partitions, heads packed 2/tile) and
     V_aug[t,(h,d|1)] (ones column folded in for softmax sums) via fp32r
     matmuls (1 cyc/row at N>=256; measured rel err 1.5e-4 at K=1024).
  2. Flash-style attention per (head-pair, q-slot): S^T = K @ Q^T with
     2-head row-packed matmuls (K=64 via tile_position), batched exp on
     ScalarE over [128,1024] PSUM spans, causal mask-mul on DVE for the
     last 4 t'-tiles of each slot, PV accumulation O^T = V_aug.T @ P^T
     (M=65: row 64 = softmax denominators l). Normalize with fast DVE
     reciprocal + GpSimd partition-broadcast.
  3. Output projection Y = O^T.T @ Wproj + bias_eff, where
     bias_eff = bproj + bv @ Wproj (V-bias folded in exactly since
     softmax rows sum to 1).

Host: slices/transposes inputs, precomputes masks + effective bias,
reassembles the 8 per-core [1024,1024] outputs into [4,2048,1024].
"""
import os
import numpy as np

import concourse.bacc as bacc
import concourse.mybir as mybir
import concourse.tile as tile
from concourse.bass_utils import run_bass_kernel_spmd

B, T, C, H, D = 4, 2048, 1024, 16, 64
QC = 256                      # q-chunk width
NSLOT = 4                     # q-chunks per core
OWN = [[0, 3, 4, 7], [1, 2, 5, 6]]   # global q-chunk ids per parity, slot order
EXT = [4, 8, 12, 16]          # padded t'-tile (128) extent per slot
F32 = mybir.dt.float32
F32R = mybir.dt.float32r
VA_W = H * (D + 1)            # 1040: V_aug cols = 16 heads x (64 + ones)

_cache = {}


def _build():
    nc = bacc.Bacc("TRN2", target_bir_lowering=False, debug=False,
                   enable_asserts=False, num_devices=8)
    dt_in = {}
    def din(name, shape):
        dt_in[name] = nc.dram_tensor(name, list(shape), F32, kind="ExternalInput").ap()
        return dt_in[name]

    xt_d = din("xt", (C, T))            # x[b].T
    xq_d = din("xq", (C, NSLOT * QC))   # own q columns of xt
    wq_d = din("wq", (C, C))            # pre-scaled by 1/8
    wk_d = din("wk", (C, C))
    wv_d = din("wv", (C, C))
    wp_d = din("wp", (C, C))
    bq_d = din("bq", (8, 128, 1))
    bk_d = din("bk", (8, 128, 1))
    bpeb_d = din("bpeb", (128, C))      # bproj_eff broadcast to 128 partitions
    mk_d = din("masks", (NSLOT, 4, 128, QC))
    y_d = nc.dram_tensor("y", [NSLOT * QC, C], F32, kind="ExternalOutput").ap()
    qt_d = nc.dram_tensor("qt", [C, NSLOT * QC], F32R).ap()
    ot_d = nc.dram_tensor("ot", [C, NSLOT * QC], F32R).ap()

    bypass = mybir.AluOpType.bypass
    mult = mybir.AluOpType.mult
    add = mybir.AluOpType.add
    EXP = mybir.ActivationFunctionType.Exp

    with tile.TileContext(nc) as tc:
        # ---------------- persistent K^T tiles --------------------------
        ktp = tc.alloc_tile_pool(name="ktp", bufs=1)
        KT = [ktp.tile([128, T], F32R, name=f"kt{j}", tag=f"kt{j}") for j in range(8)]

        # ---------------- phase 1a: K^T and Q^T -------------------------
        with tc.tile_pool(name="p1a", bufs=1) as wpool, \
             tc.tile_pool(name="p1ax", bufs=2) as xsp, \
             tc.tile_pool(name="p1ae", bufs=3) as evp, \
             tc.tile_pool(name="p1ap", bufs=1, space="PSUM") as ps1:
            wkc = [wpool.tile([128, C], F32R, name=f"wkc{c}", tag=f"wkc{c}") for c in range(8)]
            wqc = [wpool.tile([128, C], F32R, name=f"wqc{c}", tag=f"wqc{c}") for c in range(8)]
            bks = [wpool.tile([128, 1], F32, name=f"bks{j}", tag=f"bks{j}") for j in range(8)]
            bqs = [wpool.tile([128, 1], F32, name=f"bqs{j}", tag=f"bqs{j}") for j in range(8)]
            for c in range(8):
                nc.sync.dma_start(out=wkc[c][:], in_=wk_d[128*c:128*(c+1), :].bitcast(F32R))
                nc.sync.dma_start(out=wqc[c][:], in_=wq_d[128*c:128*(c+1), :].bitcast(F32R))
                nc.sync.dma_start(out=bks[c][:], in_=bk_d[c])
                nc.sync.dma_start(out=bqs[c][:], in_=bq_d[c])
            # K^T: 4 t-slabs of 512
            for slab in range(4):
                xts = []
                for c in range(8):
                    xt_t = xsp.tile([128, 512], F32R, name=f"xts{c}", tag=f"xts{c}")
                    nc.sync.dma_start(out=xt_t[:], in_=xt_d[128*c:128*(c+1), 512*slab:512*(slab+1)].bitcast(F32R))
                    xts.append(xt_t)
                pks = [ps1.tile([128, 512], F32, name=f"pk{j}", tag=f"pk{j}") for j in range(8)]
                for c in range(8):
                    for j in range(8):
                        nc.tensor.matmul(out=pks[j][:], lhsT=wkc[c][:, 128*j:128*(j+1)],
                                         rhs=xts[c][:], start=(c == 0), stop=(c == 7))
                for j in range(8):
                    nc.vector.tensor_scalar_add(out=KT[j][:, 512*slab:512*(slab+1)],
                                                in0=pks[j][:], scalar1=bks[j][:])
            # Q^T: 2 t-slabs of 512 over own columns
            for slab in range(2):
                xqs = []
                for c in range(8):
                    xq_t = xsp.tile([128, 512], F32R, name=f"xts{c}", tag=f"xts{c}")
                    nc.sync.dma_start(out=xq_t[:], in_=xq_d[128*c:128*(c+1), 512*slab:512*(slab+1)].bitcast(F32R))
                    xqs.append(xq_t)
                pqs = [ps1.tile([128, 512], F32, name=f"pk{j}", tag=f"pk{j}") for j in range(8)]
                for c in range(8):
                    for j in range(8):
                        nc.tensor.matmul(out=pqs[j][:], lhsT=wqc[c][:, 128*j:128*(j+1)],
                                         rhs=xqs[c][:], start=(c == 0), stop=(c == 7))
                for j in range(8):
                    qsb = evp.tile([128, 512], F32R, name="qsb", tag="qsb")
                    nc.vector.tensor_scalar_add(out=qsb[:], in0=pqs[j][:], scalar1=bqs[j][:])
                    nc.sync.dma_start(out=qt_d[128*j:128*(j+1), 512*slab:512*(slab+1)], in_=qsb[:])

        # ---------------- phase 1b: V_aug -------------------------------
        vap = tc.alloc_tile_pool(name="vap", bufs=1)
        VA = [vap.tile([128, VA_W], F32R, name=f"va{g}", tag=f"va{g}") for g in range(16)]
        with tc.tile_pool(name="p1b", bufs=1) as wvp, \
             tc.tile_pool(name="p1bx", bufs=1) as xsp2, \
             tc.tile_pool(name="p1bp", bufs=1, space="PSUM") as ps2:
            wvc = [wvp.tile([128, C], F32R, name=f"wvc{c}", tag=f"wvc{c}") for c in range(8)]
            for c in range(8):
                nc.sync.dma_start(out=wvc[c][:], in_=wv_d[128*c:128*(c+1), :].bitcast(F32R))
            ones16 = wvp.tile([128, H], F32, name="ones16", tag="ones16")
            nc.vector.memset(ones16[:], 1.0)
            ones16_3d = ones16[:].unsqueeze(2)
            for g in range(16):
                dst1 = VA[g][:].rearrange("p (h d) -> p h d", d=D+1)[:, :, D:D+1]
                nc.vector.tensor_copy(out=dst1, in_=ones16_3d)
            for slab in range(4):
                xts2 = []
                for c in range(8):
                    xv_t = xsp2.tile([128, 512], F32R, name=f"xv{c}", tag=f"xv{c}")
                    nc.sync.dma_start(out=xv_t[:], in_=xt_d[128*c:128*(c+1), 512*slab:512*(slab+1)].bitcast(F32R))
                    xts2.append(xv_t)
                pvs = [ps2.tile([128, 512], F32, name=f"pv{u}", tag=f"pv{u}") for u in range(8)]
                for c in range(8):
                    for tt in range(4):
                        for jc in range(2):
                            nc.tensor.matmul(out=pvs[tt*2+jc][:],
                                             lhsT=xts2[c][:, 128*tt:128*(tt+1)],
                                             rhs=wvc[c][:, 512*jc:512*(jc+1)],
                                             start=(c == 0), stop=(c == 7))
                for tt in range(4):
                    g = 4*slab + tt
                    for jc in range(2):
                        dst = VA[g][:, 520*jc:520*(jc+1)].rearrange("p (h d) -> p h d", d=D+1)[:, :, 0:D]
                        src = pvs[tt*2+jc][:].rearrange("p (h d) -> p h d", d=D)
                        nc.vector.tensor_copy(out=dst, in_=src)

        # ---------------- phase 2: attention ----------------------------
        with tc.tile_pool(name="mkp", bufs=1) as mkp, \
             tc.tile_pool(name="qrp", bufs=3) as qrp, \
             tc.tile_pool(name="ptp", bufs=3) as ptp, \
             tc.tile_pool(name="smp", bufs=2) as smp, \
             tc.tile_pool(name="p2p", bufs=1, space="PSUM") as psa:
            MK = []
            for s in range(NSLOT):
                row = []
                for mi in range(4):
                    mt = mkp.tile([128, QC], F32R, name=f"mk{s}{mi}", tag=f"mk{s}{mi}")
                    nc.sync.dma_start(out=mt[:], in_=mk_d[s, mi].bitcast(F32R))
                    row.append(mt)
                MK.append(row)
            for s in range(NSLOT):
                E = EXT[s]
                for j in range(8):
                    qr = qrp.tile([128, QC], F32R, name="qr", tag="qr")
                    nc.sync.dma_start(out=qr[:], in_=qt_d[128*j:128*(j+1), QC*s:QC*(s+1)])
                    oa = psa.tile([65, QC], F32, name="oa", tag="oa", bufs=2)
                    ob = psa.tile([65, QC], F32, name="ob", tag="ob", bufs=2)
                    for g in range(E // 2):
                        ss = psa.tile([128, 4*QC], F32, name="ss", tag="ss", bufs=2)
                        for u in range(2):
                            m = 2*g + u
                            nc.tensor.matmul(out=ss[:, QC*u:QC*(u+1)],
                                             lhsT=KT[j][0:64, 128*m:128*(m+1)],
                                             rhs=qr[0:64, :], tile_position=(0, 0),
                                             start=True, stop=True)
                            nc.tensor.matmul(out=ss[:, 2*QC+QC*u:2*QC+QC*(u+1)],
                                             lhsT=KT[j][64:128, 128*m:128*(m+1)],
                                             rhs=qr[64:128, :], tile_position=(64, 0),
                                             start=True, stop=True)
                        pt = ptp.tile([128, 4*QC], F32R, name="pt", tag="pt")
                        nc.scalar.activation(out=pt[:], in_=ss[:], func=EXP)
                        for u in range(2):
                            m = 2*g + u
                            for half, h in ((0, 2*j), (1, 2*j + 1)):
                                pcol = (2*half + u) * QC
                                psl = pt[:, pcol:pcol+QC]
                                if m >= E - 4:
                                    nc.vector.scalar_tensor_tensor(
                                        out=psl, in0=psl, scalar=0.0, in1=MK[s][m-(E-4)][:],
                                        op0=bypass, op1=mult)
                                nc.tensor.matmul(out=(oa if half == 0 else ob)[:],
                                                 lhsT=VA[m][:, 65*h:65*(h+1)],
                                                 rhs=psl,
                                                 start=(m == 0), stop=(m == E - 1))
                    # normalize: r = 1/l, broadcast, multiply; write O^T
                    for half, (acc, h) in enumerate(((oa, 2*j), (ob, 2*j + 1))):
                        lsb = smp.tile([1, QC], F32, name="lsb", tag=f"lsb{half}")
                        nc.vector.tensor_copy(out=lsb[:], in_=acc[64:65, :])
                        rsb = smp.tile([1, QC], F32, name="rsb", tag=f"rsb{half}")
                        nc.vector.reciprocal_approx_fast(rsb[:], lsb[:])
                        rbb = smp.tile([64, QC], F32, name="rbb", tag=f"rbb{half}")
                        nc.gpsimd.partition_broadcast(rbb[:], rsb[:])
                        osb = smp.tile([64, QC], F32R, name="osb", tag=f"osb{half}")
                        nc.vector.scalar_tensor_tensor(out=osb[:], in0=acc[0:64, :],
                                                       scalar=0.0, in1=rbb[:],
                                                       op0=bypass, op1=mult)
                        nc.sync.dma_start(out=ot_d[64*h:64*(h+1), QC*s:QC*(s+1)], in_=osb[:])

        # ---------------- phase 3: output projection --------------------
        with tc.tile_pool(name="p3w", bufs=1) as wpp, \
             tc.tile_pool(name="p3o", bufs=2) as lop, \
             tc.tile_pool(name="p3y", bufs=3) as yp, \
             tc.tile_pool(name="p3p", bufs=2, space="PSUM") as ps3:
            wpc = [wpp.tile([128, C], F32R, name=f"wpc{c}", tag=f"wpc{c}") for c in range(8)]
            bpeb = wpp.tile([128, C], F32, name="bpeb", tag="bpeb")
            nc.sync.dma_start(out=bpeb[:], in_=bpeb_d[:])
            for c in range(8):
                nc.sync.dma_start(out=wpc[c][:], in_=wp_d[128*c:128*(c+1), :].bitcast(F32R))
            for ti in range(8):
                lots = []
                for c in range(8):
                    lot = lop.tile([128, 128], F32R, name=f"lot{c}", tag=f"lot{c}")
                    nc.sync.dma_start(out=lot[:], in_=ot_d[128*c:128*(c+1), 128*ti:128*(ti+1)])
                    lots.append(lot)
                for jc in range(2):
                    py = ps3.tile([128, 512], F32, name="py", tag="py")
                    for c in range(8):
                        nc.tensor.matmul(out=py[:], lhsT=lots[c][:],
                                         rhs=wpc[c][:, 512*jc:512*(jc+1)],
                                         start=(c == 0), stop=(c == 7))
                    ysb = yp.tile([128, 512], F32, name="ysb", tag="ysb")
                    nc.vector.scalar_tensor_tensor(out=ysb[:], in0=py[:], scalar=0.0,
                                                   in1=bpeb[:, 512*jc:512*(jc+1)],
                                                   op0=bypass, op1=add)
                    nc.sync.dma_start(out=y_d[128*ti:128*(ti+1), 512*jc:512*(jc+1)], in_=ysb[:])
        vap.release()
        ktp.release()

    nc.compile()
    return nc


def _get_nc():
    if "nc" not in _cache:
        _cache["nc"] = _build()
    return _cache["nc"]


def _host_prep(x, Wqkv, bqkv, Wproj, bproj):
    x = np.ascontiguousarray(np.asarray(x, dtype=np.float32))
    Wqkv = np.asarray(Wqkv, dtype=np.float32)
    bqkv = np.asarray(bqkv, dtype=np.float32)
    Wproj = np.ascontiguousarray(np.asarray(Wproj, dtype=np.float32))
    bproj = np.asarray(bproj, dtype=np.float32)

    wq = np.ascontiguousarray(Wqkv[:, :C] * np.float32(0.125))
    wk = np.ascontiguousarray(Wqkv[:, C:2*C])
    wv = np.ascontiguousarray(Wqkv[:, 2*C:])
    bq8 = (bqkv[:C] * np.float32(0.125)).reshape(8, 128, 1).copy()
    bk8 = bqkv[C:2*C].reshape(8, 128, 1).copy()
    bv = bqkv[2*C:]
    bpe = (bproj.astype(np.float64) + bv.astype(np.float64) @ Wproj.astype(np.float64)).astype(np.float32)
    bpeb = np.ascontiguousarray(np.broadcast_to(bpe, (128, C)))

    pidx = np.arange(128)[:, None]
    fidx = np.arange(QC)[None, :]
    masks = []
    for par in range(2):
        mk = np.zeros((NSLOT, 4, 128, QC), dtype=np.float32)
        for s, cchunk in enumerate(OWN[par]):
            for mi in range(4):
                g = EXT[s] - 4 + mi
                mk[s, mi] = ((128*g + pidx) <= (QC*cchunk + fidx)).astype(np.float32)
        masks.append(mk)

    in_maps = []
    for core in range(8):
        b, par = core // 2, core % 2
        xt = np.ascontiguousarray(x[b].T)
        xq = np.ascontiguousarray(
            np.concatenate([xt[:, QC*c:QC*(c+1)] for c in OWN[par]], axis=1))
        in_maps.append(dict(xt=xt, xq=xq, wq=wq, wk=wk, wv=wv, wp=Wproj,
                            bq=bq8, bk=bk8, bpeb=bpeb, masks=masks[par]))
    return in_maps


def kernel(x, Wqkv, bqkv, Wproj, bproj):
    nc = _get_nc()
    in_maps = _host_prep(x, Wqkv, bqkv, Wproj, bproj)
    trace = bool(os.environ.get("BASS_TRACE"))
    res = run_bass_kernel_spmd(nc, in_maps, list(range(8)), trace=trace)
    _cache["last_exec_time_ns"] = res.exec_time_ns
    _cache["last_res"] = res
    out = np.empty((B, T, C), dtype=np.float32)
    for core in range(8):
        b, par = core // 2, core % 2
        y = res.results[core]["y"]
        for s, cchunk in enumerate(OWN[par]):
            out[b, QC*cchunk:QC*(cchunk+1)] = y[QC*s:QC*(s+1)]
    return out



# revision 5
# speedup vs baseline: 1.1518x; 1.1518x over previous
"""Causal self-attention on 8 TRN2 NeuronCores (Bass/Tile, SPMD).

Problem: B=4, T=2048, C=1024, H=16, D=64, fp32 in/out.

Sharding: core i = (batch b=i//2, parity p=i%2). Each core computes ALL 16
heads for 8 of the 16 128-wide query tiles of its batch: parity 0 owns
global q-tiles {0,3,4,7,8,11,12,15}, parity 1 owns {1,2,5,6,9,10,13,14}.
Sorted by causal extent, both parities pad to the SAME per-slot key-tile
extents E = [2,4,6,8,10,12,14,16] -> every core runs the IDENTICAL
instruction stream (SPMD); causality handled by host-supplied mask data
(mask m is triangular/ones/zeros depending on parity). K/V are computed
for the full sequence on both cores of a batch (duplication beats
communication here).

All tensor data is bf16 (PSUM accumulation fp32): same 1 cyc/row PE
stream rate as fp32r but half the SBUF/DMA traffic, half LDWEIGHTS cost,
4x DVE rate on the bf16 mask multiplies, and no N>=256 matmul
restriction (enables exact extents).

Per-core pipeline:
  1. K^T[d,t] (d on partitions, 2 heads/tile) via j-major matmuls with
     lhsT(weights) reused across t; V_aug[t,(h,d|1)] (ones column folded
     for softmax sums) reusing the same x tiles; Q^T likewise from the
     core's own (slot-sorted) query columns. PSUM evacuation (+bias) on
     the otherwise-idle Act engine.
  2. Attention per (head-pair j, pass, key-tile pair): the q-tiles
     needing key-tile m form a contiguous SUFFIX of the slot-sorted Q
     columns, so S^T(m) = K_m^T Q_suffix is ONE long matmul (N=512..128)
     per (half, m); batched exp on ScalarE over [128, 2N] PSUM spans;
     per-pair diagonal masks on DVE (bf16, 4x mode); PV accumulated into
     per-pass [65,512] PSUM accumulators (row 64 = softmax denominators),
     software-pipelined one pair behind S so PE never waits on exp.
     Two passes (slots 0-3, 4-7) keep PSUM within 8 banks.
     Normalize with DVE reciprocal + GpSimd broadcast; O^T stays in SBUF.
  3. Output projection Y = O^T.T @ Wproj + bias_eff directly from SBUF
     (bias_eff = bproj + bv @ Wproj, exact since softmax rows sum to 1).

Host: slices/transposes/casts inputs to bf16, precomputes masks +
effective bias, reassembles the 8 per-core [1024,1024] outputs.
"""
import os
import numpy as np
import ml_dtypes

import concourse.bacc as bacc
import concourse.mybir as mybir
import concourse.tile as tile
from concourse.bass_utils import run_bass_kernel_spmd

B, T, C, H, D = 4, 2048, 1024, 16, 64
F32 = mybir.dt.float32
BF16 = mybir.dt.bfloat16
NPBF = ml_dtypes.bfloat16
VA_W = H * (D + 1)            # 1040: V_aug cols = 16 heads x (64 + ones)
OWNT = [[0, 3, 4, 7, 8, 11, 12, 15], [1, 2, 5, 6, 9, 10, 13, 14]]
EXT = [2, 4, 6, 8, 10, 12, 14, 16]   # padded key-tile extent per slot

_cache = {}


def _build():
    nc = bacc.Bacc("TRN2", target_bir_lowering=False, debug=False,
                   enable_asserts=False, num_devices=8)
    def din(name, shape, dt=BF16):
        return nc.dram_tensor(name, list(shape), dt, kind="ExternalInput").ap()

    xt_d = din("xt", (C, T))            # x[b].T
    xq_d = din("xq", (C, 1024))         # own q columns of xt, slot-sorted
    wq_d = din("wq", (C, C))            # pre-scaled by 1/8
    wk_d = din("wk", (C, C))
    wv_d = din("wv", (C, C))
    wp_d = din("wp", (C, C))
    bq_d = din("bq", (8, 128, 1), F32)  # pre-scaled by 1/8
    bk_d = din("bk", (8, 128, 1), F32)
    bpeb_d = din("bpeb", (128, C), F32)  # bproj_eff broadcast to 128 partitions
    mk_d = din("masks", (128, 16 * 128))
    y_d = nc.dram_tensor("y", [1024, C], F32, kind="ExternalOutput").ap()

    bypass = mybir.AluOpType.bypass
    mult = mybir.AluOpType.mult
    add = mybir.AluOpType.add
    EXP = mybir.ActivationFunctionType.Exp

    with tile.TileContext(nc) as tc:
        # ---------------- persistent tiles ------------------------------
        per = tc.alloc_tile_pool(name="per", bufs=1)
        KT = [per.tile([128, T], BF16, name=f"kt{j}", tag=f"kt{j}") for j in range(8)]
        VA = [per.tile([128, VA_W], BF16, name=f"va{m}", tag=f"va{m}") for m in range(16)]
        QT = [per.tile([128, 1024], BF16, name=f"qt{j}", tag=f"qt{j}") for j in range(8)]
        OT = [per.tile([128, 1024], BF16, name=f"ot{j}", tag=f"ot{j}") for j in range(8)]
        MK = per.tile([128, 16 * 128], BF16, name="mk", tag="mk")
        bks = [per.tile([128, 1], F32, name=f"bks{j}", tag=f"bks{j}") for j in range(8)]
        bqs = [per.tile([128, 1], F32, name=f"bqs{j}", tag=f"bqs{j}") for j in range(8)]
        bpeb = per.tile([128, C], F32, name="bpeb", tag="bpeb")
        ones16 = per.tile([128, H], BF16, name="ones16", tag="ones16")

        # -------- phase 1: K^T, V_aug, Q^T ------------------------------
        wkvp = tc.alloc_tile_pool(name="wkvp", bufs=1)
        wkc = [wkvp.tile([128, C], BF16, name=f"wkc{c}", tag=f"wkc{c}") for c in range(8)]
        wvc = [wkvp.tile([128, C], BF16, name=f"wvc{c}", tag=f"wvc{c}") for c in range(8)]
        wqp = tc.alloc_tile_pool(name="wqp", bufs=1)
        wqc = [wqp.tile([128, C], BF16, name=f"wqc{c}", tag=f"wqc{c}") for c in range(8)]
        xqp = tc.alloc_tile_pool(name="xqp", bufs=1)
        xqc = [xqp.tile([128, 1024], BF16, name=f"xqc{c}", tag=f"xqc{c}") for c in range(8)]
        with tc.tile_pool(name="xsp", bufs=1) as xsp, \
             tc.tile_pool(name="ps1", bufs=3, space="PSUM") as ps1:
            # DMAs in priority order: K weights + first x slab first.
            for c in range(8):
                nc.sync.dma_start(out=wkc[c][:], in_=wk_d[128*c:128*(c+1), :])
            for j in range(8):
                nc.sync.dma_start(out=bks[j][:], in_=bk_d[j])
            xts0 = []
            for c in range(8):
                xt_t = xsp.tile([128, 1024], BF16, name=f"xts{c}", tag=f"xts{c}")
                nc.sync.dma_start(out=xt_t[:], in_=xt_d[128*c:128*(c+1), 0:1024])
                xts0.append(xt_t)
            for c in range(8):
                nc.sync.dma_start(out=wvc[c][:], in_=wv_d[128*c:128*(c+1), :])
            # prefetch: Q weights/inputs, masks, misc, then second x slab
            for c in range(8):
                nc.sync.dma_start(out=wqc[c][:], in_=wq_d[128*c:128*(c+1), :])
                nc.sync.dma_start(out=xqc[c][:], in_=xq_d[128*c:128*(c+1), :])
            for j in range(8):
                nc.sync.dma_start(out=bqs[j][:], in_=bq_d[j])
            nc.sync.dma_start(out=MK[:], in_=mk_d[:])
            nc.sync.dma_start(out=bpeb[:], in_=bpeb_d[:])
            xts1 = []
            for c in range(8):
                xt_t = xsp.tile([128, 1024], BF16, name=f"xts{c}", tag=f"xts{c}")
                nc.sync.dma_start(out=xt_t[:], in_=xt_d[128*c:128*(c+1), 1024:2048])
                xts1.append(xt_t)

            # ones columns of V_aug
            nc.vector.memset(ones16[:], 1.0)
            ones16_3d = ones16[:].unsqueeze(2)
            for m in range(16):
                dst1 = VA[m][:].rearrange("p (h d) -> p h d", d=D+1)[:, :, D:D+1]
                nc.vector.tensor_copy(out=dst1, in_=ones16_3d)

            for ss, xts in ((0, xts0), (1, xts1)):
                # K^T for this t-superslab: j-major, lhsT reused over 2 subs
                for j in range(8):
                    pk = ps1.tile([128, 1024], F32, name="pk", tag="pk")
                    for c in range(8):
                        for sub in range(2):
                            nc.tensor.matmul(out=pk[:, 512*sub:512*(sub+1)],
                                             lhsT=wkc[c][:, 128*j:128*(j+1)],
                                             rhs=xts[c][:, 512*sub:512*(sub+1)],
                                             start=(c == 0), stop=(c == 7))
                    nc.scalar.add(out=KT[j][:, 1024*ss:1024*(ss+1)], in_=pk[:],
                                  add=bks[j][:])
                # V for this superslab: 8 t-tiles (m = 8*ss + tt)
                for tt in range(8):
                    pv = ps1.tile([128, 1024], F32, name="pv", tag="pk")
                    for c in range(8):
                        for jc in range(2):
                            nc.tensor.matmul(out=pv[:, 512*jc:512*(jc+1)],
                                             lhsT=xts[c][:, 128*tt:128*(tt+1)],
                                             rhs=wvc[c][:, 512*jc:512*(jc+1)],
                                             start=(c == 0), stop=(c == 7))
                    dst = VA[8*ss+tt][:].rearrange("p (h d) -> p h d", d=D+1)[:, :, 0:D]
                    src = pv[:].rearrange("p (h d) -> p h d", d=D)
                    nc.scalar.copy(out=dst, in_=src)
            # Q^T (own columns, slot-sorted)
            for j in range(8):
                pq = ps1.tile([128, 1024], F32, name="pq", tag="pk")
                for c in range(8):
                    for sub in range(2):
                        nc.tensor.matmul(out=pq[:, 512*sub:512*(sub+1)],
                                         lhsT=wqc[c][:, 128*j:128*(j+1)],
                                         rhs=xqc[c][:, 512*sub:512*(sub+1)],
                                         start=(c == 0), stop=(c == 7))
                nc.scalar.add(out=QT[j][:, :], in_=pq[:], add=bqs[j][:])
        xqp.release()
        wqp.release()
        wkvp.release()

        # ---------------- phase 2: attention ----------------------------
        wpp = tc.alloc_tile_pool(name="wpp", bufs=1)
        wpc = [wpp.tile([128, C], BF16, name=f"wpc{c}", tag=f"wpc{c}") for c in range(8)]
        with tc.tile_pool(name="att", bufs=1) as att, \
             tc.tile_pool(name="ps2", bufs=1, space="PSUM") as ps2:
            for c in range(8):
                nc.sync.dma_start(out=wpc[c][:], in_=wp_d[128*c:128*(c+1), :])
            for j in range(8):
                for p in range(2):          # pass: slots 4p..4p+3
                    npair = 4 + 4 * p       # key-tile pairs in this pass
                    mlast = 8 + 8 * p - 1
                    acc = [ps2.tile([65, 512], F32, name=f"acc{p}{h}",
                                    tag=f"acc{p}{h}") for h in range(2)]
                    pend = None             # PV args deferred by one pair
                    for g in range(npair):
                        m0, m1 = 2*g, 2*g + 1
                        so = max(0, g - 4*p)           # suffix start slot (pass-local)
                        qc0 = 512*p + 128*so           # suffix start col in QT
                        N = 512 - 128*so
                        masked = (g >= 4*p)            # diagonal slot in this pass?
                        cur = []
                        for half in range(2):
                            r0, r1 = 64*half, 64*(half+1)
                            ss_t = ps2.tile([128, 1024], F32, name="ss", tag="ss",
                                            bufs=2)
                            for u, m in ((0, m0), (1, m1)):
                                # split dst at PSUM bank boundaries (512 cols)
                                c0 = N * u
                                while c0 < N * (u + 1):
                                    c1 = min(N * (u + 1), (c0 // 512 + 1) * 512)
                                    nc.tensor.matmul(out=ss_t[:, c0:c1],
                                                     lhsT=KT[j][r0:r1, 128*m:128*(m+1)],
                                                     rhs=QT[j][r0:r1,
                                                               qc0 + c0 - N*u:
                                                               qc0 + c1 - N*u],
                                                     tile_position=(r0, 0),
                                                     start=True, stop=True)
                                    c0 = c1
                            pt = att.tile([128, 1024], BF16, name="pt", tag="pt",
                                          bufs=5)
                            nc.scalar.activation(out=pt[:, 0:2*N], in_=ss_t[:, 0:2*N],
                                                 func=EXP)
                            if masked:
                                for u, m in ((0, m0), (1, m1)):
                                    nc.vector.scalar_tensor_tensor(
                                        out=pt[:, N*u:N*u+128], in0=pt[:, N*u:N*u+128],
                                        scalar=0.0, in1=MK[:, 128*m:128*(m+1)],
                                        op0=bypass, op1=mult)
                            cur.append((pt, N, so, (m0, m1), half))
                        if pend is not None:
                            for (ppt, pN, pso, pms, phalf) in pend:
                                hh = 2*j + phalf
                                for u, m in ((0, pms[0]), (1, pms[1])):
                                    nc.tensor.matmul(out=acc[phalf][:, 128*pso:512],
                                                     lhsT=VA[m][:, 65*hh:65*(hh+1)],
                                                     rhs=ppt[:, pN*u:pN*(u+1)],
                                                     start=(m == 0), stop=(m == mlast),
                                                     skip_group_check=True)
                        pend = cur
                    for (ppt, pN, pso, pms, phalf) in pend:
                        hh = 2*j + phalf
                        for u, m in ((0, pms[0]), (1, pms[1])):
                            nc.tensor.matmul(out=acc[phalf][:, 128*pso:512],
                                             lhsT=VA[m][:, 65*hh:65*(hh+1)],
                                             rhs=ppt[:, pN*u:pN*(u+1)],
                                             start=(m == 0), stop=(m == mlast),
                                             skip_group_check=True)
                    # normalize: r = 1/l, broadcast, multiply; write O^T (SBUF)
                    for half in range(2):
                        lsb = att.tile([1, 512], F32, name="lsb", tag=f"lsb{half}",
                                       bufs=2)
                        nc.vector.tensor_copy(out=lsb[:], in_=acc[half][64:65, :])
                        rsb = att.tile([1, 512], F32, name="rsb", tag=f"rsb{half}",
                                       bufs=2)
                        nc.vector.reciprocal_approx_fast(rsb[:], lsb[:])
                        rbb = att.tile([64, 512], F32, name="rbb", tag=f"rbb{half}",
                                       bufs=2)
                        nc.gpsimd.partition_broadcast(rbb[:], rsb[:])
                        nc.vector.scalar_tensor_tensor(
                            out=OT[j][64*half:64*(half+1), 512*p:512*(p+1)],
                            in0=acc[half][0:64, :], scalar=0.0, in1=rbb[:],
                            op0=bypass, op1=mult)

        # ---------------- phase 3: output projection --------------------
        with tc.tile_pool(name="p3y", bufs=3) as yp, \
             tc.tile_pool(name="ps3", bufs=1, space="PSUM") as ps3:
            for ti in range(8):
                py = [ps3.tile([128, 512], F32, name=f"py{jc}", tag=f"py{jc}",
                               bufs=2) for jc in range(2)]
                for ci in range(8):
                    for jc in range(2):
                        nc.tensor.matmul(out=py[jc][:],
                                         lhsT=OT[ci][:, 128*ti:128*(ti+1)],
                                         rhs=wpc[ci][:, 512*jc:512*(jc+1)],
                                         start=(ci == 0), stop=(ci == 7))
                for jc in range(2):
                    ysb = yp.tile([128, 512], F32, name="ysb", tag="ysb")
                    nc.vector.scalar_tensor_tensor(out=ysb[:], in0=py[jc][:],
                                                   scalar=0.0,
                                                   in1=bpeb[:, 512*jc:512*(jc+1)],
                                                   op0=bypass, op1=add)
                    nc.sync.dma_start(out=y_d[128*ti:128*(ti+1), 512*jc:512*(jc+1)],
                                      in_=ysb[:])
        wpp.release()
        per.release()

    nc.compile()
    return nc


def _get_nc():
    if "nc" not in _cache:
        _cache["nc"] = _build()
    return _cache["nc"]


def _host_prep(x, Wqkv, bqkv, Wproj, bproj):
    x = np.ascontiguousarray(np.asarray(x, dtype=np.float32))
    Wqkv = np.asarray(Wqkv, dtype=np.float32)
    bqkv = np.asarray(bqkv, dtype=np.float32)
    Wproj = np.ascontiguousarray(np.asarray(Wproj, dtype=np.float32))
    bproj = np.asarray(bproj, dtype=np.float32)

    wq = np.ascontiguousarray(Wqkv[:, :C] * np.float32(0.125)).astype(NPBF)
    wk = np.ascontiguousarray(Wqkv[:, C:2*C]).astype(NPBF)
    wv = np.ascontiguousarray(Wqkv[:, 2*C:]).astype(NPBF)
    wp = Wproj.astype(NPBF)
    bq8 = (bqkv[:C] * np.float32(0.125)).reshape(8, 128, 1).copy()
    bk8 = bqkv[C:2*C].reshape(8, 128, 1).copy()
    bv = bqkv[2*C:]
    bpe = (bproj.astype(np.float64) + bv.astype(np.float64) @ Wproj.astype(np.float64)).astype(np.float32)
    bpeb = np.ascontiguousarray(np.broadcast_to(bpe, (128, C)))

    ridx = np.arange(128)[:, None]
    cidx = np.arange(128)[None, :]
    tri = (ridx <= cidx)
    masks = []
    for par in range(2):
        mk = np.zeros((128, 16 * 128), dtype=NPBF)
        for m in range(16):
            g = OWNT[par][m // 2]
            if m < g:
                mk[:, 128*m:128*(m+1)] = 1
            elif m == g:
                mk[:, 128*m:128*(m+1)] = tri
        masks.append(mk)

    in_maps = []
    for core in range(8):
        b, par = core // 2, core % 2
        xt = np.ascontiguousarray(x[b].T.astype(NPBF))
        xq = np.ascontiguousarray(
            np.concatenate([xt[:, 128*g:128*(g+1)] for g in OWNT[par]], axis=1))
        in_maps.append(dict(xt=xt, xq=xq, wq=wq, wk=wk, wv=wv, wp=wp,
                            bq=bq8, bk=bk8, bpeb=bpeb, masks=masks[par]))
    return in_maps


def kernel(x, Wqkv, bqkv, Wproj, bproj):
    nc = _get_nc()
    in_maps = _host_prep(x, Wqkv, bqkv, Wproj, bproj)
    trace = bool(os.environ.get("BASS_TRACE"))
    res = run_bass_kernel_spmd(nc, in_maps, list(range(8)), trace=trace)
    _cache["last_exec_time_ns"] = res.exec_time_ns
    _cache["last_res"] = res
    out = np.empty((B, T, C), dtype=np.float32)
    for core in range(8):
        b, par = core // 2, core % 2
        y = res.results[core]["y"]
        for si, g in enumerate(OWNT[par]):
            out[b, 128*g:128*(g+1)] = y[128*si:128*(si+1)]
    return out


# revision 11
# speedup vs baseline: 1.2468x; 1.0824x over previous
"""Causal self-attention on 8 TRN2 NeuronCores (Bass/Tile, SPMD).

Problem: B=4, T=2048, C=1024, H=16, D=64, fp32 in/out.

Sharding: core i = (batch b=i//2, parity p=i%2). Each core computes ALL 16
heads for 8 of the 16 128-wide query tiles of its batch: parity 0 owns
global q-tiles {0,3,4,7,8,11,12,15}, parity 1 owns {1,2,5,6,9,10,13,14}.
Sorted by causal extent, both parities pad to the SAME per-slot key-tile
extents E = [2,4,6,8,10,12,14,16] -> every core runs the IDENTICAL
instruction stream (SPMD); causality handled by host-supplied mask data
(mask m is triangular/ones/zeros depending on parity). K/V are computed
for the full sequence on both cores of a batch (duplication beats
communication here).

All tensor data is bf16 (PSUM accumulation fp32): same 1 cyc/row PE
stream rate as fp32r but half the SBUF/DMA traffic, half LDWEIGHTS cost,
4x DVE rate on the bf16 mask multiplies, and no N>=256 matmul
restriction (enables exact extents).

Per-core pipeline:
  1. K^T[d,t] (d on partitions, 2 heads/tile) via j-major matmuls with
     lhsT(weights) reused across t; V_aug[t,(h,d|1)] (ones column folded
     for softmax sums) reusing the same x tiles; Q^T likewise from the
     core's own (slot-sorted) query columns. PSUM evacuation (+bias) on
     the otherwise-idle Act engine.
  2. Attention per (head-pair j, pass, key-tile pair): the q-tiles
     needing key-tile m form a contiguous SUFFIX of the slot-sorted Q
     columns, so S^T(m) = K_m^T Q_suffix is ONE long matmul (N=512..128)
     per (half, m); batched exp on ScalarE over [128, 2N] PSUM spans;
     per-pair diagonal masks on DVE (bf16, 4x mode); PV accumulated into
     per-pass [65,512] PSUM accumulators (row 64 = softmax denominators),
     software-pipelined one pair behind S so PE never waits on exp.
     Two passes (slots 0-3, 4-7) keep PSUM within 8 banks.
     Normalize with DVE reciprocal + GpSimd broadcast; O^T stays in SBUF.
  3. Output projection Y = O^T.T @ Wproj + bias_eff directly from SBUF
     (bias_eff = bproj + bv @ Wproj, exact since softmax rows sum to 1).

Host: slices/transposes/casts inputs to bf16, precomputes masks +
effective bias, reassembles the 8 per-core [1024,1024] outputs.
"""
import os
import numpy as np
import ml_dtypes

import concourse.bacc as bacc
import concourse.mybir as mybir
import concourse.tile as tile
from concourse.bass_utils import run_bass_kernel_spmd

B, T, C, H, D = 4, 2048, 1024, 16, 64
F32 = mybir.dt.float32
BF16 = mybir.dt.bfloat16
NPBF = ml_dtypes.bfloat16
VA_W = H * (D + 1)            # 1040: V_aug cols = 16 heads x (64 + ones)
OWNT = [[0, 3, 4, 7, 8, 11, 12, 15], [1, 2, 5, 6, 9, 10, 13, 14]]
EXT = [2, 4, 6, 8, 10, 12, 14, 16]   # padded key-tile extent per slot

_cache = {}


def _build():
    nc = bacc.Bacc("TRN2", target_bir_lowering=False, debug=False,
                   enable_asserts=False, num_devices=8)
    def din(name, shape, dt=BF16):
        return nc.dram_tensor(name, list(shape), dt, kind="ExternalInput").ap()

    xt_d = din("xt", (C, T))            # x[b].T
    xq_d = din("xq", (C, 1024))         # own q columns of xt, slot-sorted
    wq_d = din("wq", (C, C))            # pre-scaled by 1/8
    wk_d = din("wk", (C, C))
    wv_d = din("wv", (C, C))
    wp_d = din("wp", (C, C))
    bq_d = din("bq", (8, 128, 1), F32)  # pre-scaled by 1/8
    bk_d = din("bk", (8, 128, 1), F32)
    bpeb_d = din("bpeb", (128, C), F32)  # bproj_eff broadcast to 128 partitions
    mk_d = din("masks", (128, 16 * 128))
    y_d = nc.dram_tensor("y", [1024, C], F32, kind="ExternalOutput").ap()

    bypass = mybir.AluOpType.bypass
    mult = mybir.AluOpType.mult
    add = mybir.AluOpType.add
    EXP = mybir.ActivationFunctionType.Exp

    with tile.TileContext(nc) as tc:
        # ---------------- persistent tiles ------------------------------
        per = tc.alloc_tile_pool(name="per", bufs=1)
        KT = [per.tile([128, T], BF16, name=f"kt{j}", tag=f"kt{j}") for j in range(8)]
        VA = [per.tile([128, VA_W], BF16, name=f"va{m}", tag=f"va{m}") for m in range(16)]
        QT = [per.tile([128, 1024], BF16, name=f"qt{j}", tag=f"qt{j}") for j in range(8)]
        OT = [per.tile([128, 1024], BF16, name=f"ot{j}", tag=f"ot{j}") for j in range(8)]
        MK = per.tile([128, 16 * 128], BF16, name="mk", tag="mk")
        bks = [per.tile([128, 1], F32, name=f"bks{j}", tag=f"bks{j}") for j in range(8)]
        bqs = [per.tile([128, 1], F32, name=f"bqs{j}", tag=f"bqs{j}") for j in range(8)]
        bpeb = per.tile([128, C], F32, name="bpeb", tag="bpeb")
        ones16 = per.tile([128, H], BF16, name="ones16", tag="ones16")

        # -------- phase 1: K^T, V_aug, Q^T ------------------------------
        wkvp = tc.alloc_tile_pool(name="wkvp", bufs=1)
        wkc = [wkvp.tile([128, C], BF16, name=f"wkc{c}", tag=f"wkc{c}") for c in range(8)]
        wvc = [wkvp.tile([128, C], BF16, name=f"wvc{c}", tag=f"wvc{c}") for c in range(8)]
        wqp = tc.alloc_tile_pool(name="wqp", bufs=1)
        wqc = [wqp.tile([128, C], BF16, name=f"wqc{c}", tag=f"wqc{c}") for c in range(8)]
        xqp = tc.alloc_tile_pool(name="xqp", bufs=1)
        xqc = [xqp.tile([128, 1024], BF16, name=f"xqc{c}", tag=f"xqc{c}") for c in range(8)]
        with tc.tile_pool(name="xsp", bufs=1) as xsp, \
             tc.tile_pool(name="ps1", bufs=3, space="PSUM") as ps1:
            # DMAs in priority order: K weights + first x slab first.
            for c in range(8):
                nc.sync.dma_start(out=wkc[c][:], in_=wk_d[128*c:128*(c+1), :])
            for j in range(8):
                nc.sync.dma_start(out=bks[j][:], in_=bk_d[j])
            xts0 = []
            for c in range(8):
                xt_t = xsp.tile([128, 1024], BF16, name=f"xts{c}", tag=f"xts{c}")
                nc.sync.dma_start(out=xt_t[:], in_=xt_d[128*c:128*(c+1), 0:1024])
                xts0.append(xt_t)
            for c in range(8):
                nc.sync.dma_start(out=wvc[c][:], in_=wv_d[128*c:128*(c+1), :])
            # prefetch: Q weights/inputs, masks, misc, then second x slab
            for c in range(8):
                nc.sync.dma_start(out=wqc[c][:], in_=wq_d[128*c:128*(c+1), :])
                nc.sync.dma_start(out=xqc[c][:], in_=xq_d[128*c:128*(c+1), :])
            for j in range(8):
                nc.sync.dma_start(out=bqs[j][:], in_=bq_d[j])
            nc.sync.dma_start(out=MK[:], in_=mk_d[:])
            nc.sync.dma_start(out=bpeb[:], in_=bpeb_d[:])
            xts1 = []
            for c in range(8):
                xt_t = xsp.tile([128, 1024], BF16, name=f"xts{c}", tag=f"xts{c}")
                nc.sync.dma_start(out=xt_t[:], in_=xt_d[128*c:128*(c+1), 1024:2048])
                xts1.append(xt_t)

            # ones columns of V_aug
            nc.vector.memset(ones16[:], 1.0)
            ones16_3d = ones16[:].unsqueeze(2)
            for m in range(16):
                dst1 = VA[m][:].rearrange("p (h d) -> p h d", d=D+1)[:, :, D:D+1]
                nc.vector.tensor_copy(out=dst1, in_=ones16_3d)

            for ss, xts in ((0, xts0), (1, xts1)):
                # K^T for this t-superslab: j-major, lhsT reused over 2 subs
                for j in range(8):
                    pk = ps1.tile([128, 1024], F32, name="pk", tag="pk")
                    for c in range(8):
                        for sub in range(2):
                            nc.tensor.matmul(out=pk[:, 512*sub:512*(sub+1)],
                                             lhsT=wkc[c][:, 128*j:128*(j+1)],
                                             rhs=xts[c][:, 512*sub:512*(sub+1)],
                                             start=(c == 0), stop=(c == 7))
                    nc.scalar.add(out=KT[j][:, 1024*ss:1024*(ss+1)], in_=pk[:],
                                  add=bks[j][:])
                # V for this superslab: 8 t-tiles (m = 8*ss + tt)
                for tt in range(8):
                    pv = ps1.tile([128, 1024], F32, name="pv", tag="pk")
                    for c in range(8):
                        for jc in range(2):
                            nc.tensor.matmul(out=pv[:, 512*jc:512*(jc+1)],
                                             lhsT=xts[c][:, 128*tt:128*(tt+1)],
                                             rhs=wvc[c][:, 512*jc:512*(jc+1)],
                                             start=(c == 0), stop=(c == 7))
                    dst = VA[8*ss+tt][:].rearrange("p (h d) -> p h d", d=D+1)[:, :, 0:D]
                    src = pv[:].rearrange("p (h d) -> p h d", d=D)
                    nc.scalar.copy(out=dst, in_=src)
            # Q^T (own columns, slot-sorted)
            for j in range(8):
                pq = ps1.tile([128, 1024], F32, name="pq", tag="pk")
                for c in range(8):
                    for sub in range(2):
                        nc.tensor.matmul(out=pq[:, 512*sub:512*(sub+1)],
                                         lhsT=wqc[c][:, 128*j:128*(j+1)],
                                         rhs=xqc[c][:, 512*sub:512*(sub+1)],
                                         start=(c == 0), stop=(c == 7))
                nc.scalar.add(out=QT[j][:, :], in_=pq[:], add=bqs[j][:])
        xqp.release()
        wqp.release()
        wkvp.release()

        # ---------------- phase 2: attention ----------------------------
        # X sweep (pass 0, slots 0-3) for all j, then Y sweep (pass 1) with
        # the X-half of the output projection interleaved, then proj-Y.
        wpp = tc.alloc_tile_pool(name="wpp", bufs=1)
        wpc = [wpp.tile([128, C], BF16, name=f"wpc{c}", tag=f"wpc{c}") for c in range(8)]

        def emit_pair_packed(ps, att, ss_bufs, j, p, g):
            """All 4 streams (2 halves x 2 key-tiles) in one ss tile, one exp.
            Only valid when 4N <= 1024 (no PSUM bank crossing possible)."""
            m0, m1 = 2*g, 2*g + 1
            so = max(0, g - 4*p)
            qc0 = 512*p + 128*so
            N = 512 - 128*so
            masked = (g >= 4*p)
            halves = [(0, [(0, m0), (1, m1)]), (1, [(2, m0), (3, m1)])]
            ss_t = ps.tile([128, 1024], F32, name="ss", tag="ss", bufs=ss_bufs)
            for half, ums in halves:
                r0, r1 = 64*half, 64*(half+1)
                for u, m in ums:
                    nc.tensor.matmul(out=ss_t[:, N*u:N*(u+1)],
                                     lhsT=KT[j][r0:r1, 128*m:128*(m+1)],
                                     rhs=QT[j][r0:r1, qc0:qc0+N],
                                     tile_position=(r0, 0),
                                     start=True, stop=True)
            pt = att.tile([128, 1024], BF16, name="pt", tag="pt", bufs=8)
            nc.scalar.activation(out=pt[:, 0:4*N], in_=ss_t[:, 0:4*N], func=EXP)
            out = []
            for half, ums in halves:
                if masked:
                    for u, m in ums:
                        nc.vector.scalar_tensor_tensor(
                            out=pt[:, N*u:N*u+128], in0=pt[:, N*u:N*u+128],
                            scalar=0.0, in1=MK[:, 128*m:128*(m+1)],
                            op0=bypass, op1=mult)
                out.append((pt, N, so, ums, half))
            return out

        def emit_pair_unpacked(ps, att, ss_bufs, j, p, g):
            m0, m1 = 2*g, 2*g + 1
            so = max(0, g - 4*p)
            qc0 = 512*p + 128*so
            N = 512 - 128*so
            masked = (g >= 4*p)
            out = []
            for half in range(2):
                r0, r1 = 64*half, 64*(half+1)
                ss_t = ps.tile([128, 1024], F32, name="ss", tag="ss", bufs=ss_bufs)
                for u, m in ((0, m0), (1, m1)):
                    c0 = N * u
                    while c0 < N * (u + 1):
                        c1 = min(N * (u + 1), (c0 // 512 + 1) * 512)
                        nc.tensor.matmul(out=ss_t[:, c0:c1],
                                         lhsT=KT[j][r0:r1, 128*m:128*(m+1)],
                                         rhs=QT[j][r0:r1, qc0 + c0 - N*u:
                                                          qc0 + c1 - N*u],
                                         tile_position=(r0, 0),
                                         start=True, stop=True)
                        c0 = c1
                pt = att.tile([128, 1024], BF16, name="pt", tag="pt", bufs=8)
                nc.scalar.activation(out=pt[:, 0:2*N], in_=ss_t[:, 0:2*N], func=EXP)
                if masked:
                    for u, m in ((0, m0), (1, m1)):
                        nc.vector.scalar_tensor_tensor(
                            out=pt[:, N*u:N*u+128], in0=pt[:, N*u:N*u+128],
                            scalar=0.0, in1=MK[:, 128*m:128*(m+1)],
                            op0=bypass, op1=mult)
                out.append((pt, N, so, [(0, m0), (1, m1)], half))
            return out

        def emit_attn_pair(ps, att, ss_bufs, j, p, g):
            so = max(0, g - 4*p)
            N = 512 - 128*so
            if 4*N == 1024:     # packed halves land in separate PSUM banks
                return emit_pair_packed(ps, att, ss_bufs, j, p, g)
            return emit_pair_unpacked(ps, att, ss_bufs, j, p, g)

        def emit_pv(acc, j, mlast, items):
            for (ppt, pN, pso, ums, phalf) in items:
                hh = 2*j + phalf
                for u, m in ums:
                    nc.tensor.matmul(out=acc[phalf][:, 128*pso:512],
                                     lhsT=VA[m][:, 65*hh:65*(hh+1)],
                                     rhs=ppt[:, pN*u:pN*(u+1)],
                                     start=(m == 0), stop=(m == mlast),
                                     skip_group_check=True)

        def emit_norm(att, acc, j, p):
            for half in range(2):
                lsb = att.tile([1, 512], F32, name="lsb", tag=f"lsb{half}", bufs=2)
                nc.vector.tensor_copy(out=lsb[:], in_=acc[half][64:65, :])
                rsb = att.tile([1, 512], F32, name="rsb", tag=f"rsb{half}", bufs=2)
                nc.vector.reciprocal_approx_fast(rsb[:], lsb[:])
                rbb = att.tile([64, 512], F32, name="rbb", tag=f"rbb{half}", bufs=2)
                nc.gpsimd.partition_broadcast(rbb[:], rsb[:])
                nc.vector.scalar_tensor_tensor(
                    out=OT[j][64*half:64*(half+1), 512*p:512*(p+1)],
                    in0=acc[half][0:64, :], scalar=0.0, in1=rbb[:],
                    op0=bypass, op1=mult)

        LAG = 2
        with tc.tile_pool(name="att", bufs=1) as att:
            for c in range(8):
                nc.sync.dma_start(out=wpc[c][:], in_=wp_d[128*c:128*(c+1), :])
            # ---- X sweep: pass 0 for all j (deep ss pipeline) ----
            with tc.tile_pool(name="psx", bufs=1, space="PSUM") as psx:
                for j in range(8):
                    acc = [psx.tile([65, 512], F32, name=f"acc{h}", tag=f"acc{h}")
                           for h in range(2)]
                    pend = []
                    for g in range(4):
                        pend.append(emit_attn_pair(psx, att, 3, j, 0, g))
                        if len(pend) > LAG:
                            emit_pv(acc, j, 7, pend.pop(0))
                    for items in pend:
                        emit_pv(acc, j, 7, items)
                    emit_norm(att, acc, j, 0)
            # ---- Y sweep: pass 1 for all j, proj-X interleaved ----
            with tc.tile_pool(name="psy", bufs=1, space="PSUM") as psy:
                def proj_gen(tis):
                    for ti in tis:
                        py = [psy.tile([128, 512], F32, name=f"py{jc}",
                                       tag=f"py{jc}", bufs=1) for jc in range(2)]
                        for ci in range(8):
                            for jc in range(2):
                                nc.tensor.matmul(out=py[jc][:],
                                                 lhsT=OT[ci][:, 128*ti:128*(ti+1)],
                                                 rhs=wpc[ci][:, 512*jc:512*(jc+1)],
                                                 start=(ci == 0), stop=(ci == 7))
                            yield
                        for jc in range(2):
                            ysb = att.tile([128, 512], F32, name="ysb", tag="ysb",
                                           bufs=3)
                            nc.vector.scalar_tensor_tensor(
                                out=ysb[:], in0=py[jc][:], scalar=0.0,
                                in1=bpeb[:, 512*jc:512*(jc+1)],
                                op0=bypass, op1=add)
                            nc.sync.dma_start(
                                out=y_d[128*ti:128*(ti+1), 512*jc:512*(jc+1)],
                                in_=ysb[:])
                        yield
                gx = proj_gen(range(4))
                for j in range(8):
                    acc = [psy.tile([65, 512], F32, name=f"acc{h}", tag=f"acc{h}")
                           for h in range(2)]
                    pend = []
                    for g in range(8):
                        pend.append(emit_attn_pair(psy, att, 2, j, 1, g))
                        if len(pend) > LAG:
                            emit_pv(acc, j, 15, pend.pop(0))
                    for items in pend:
                        emit_pv(acc, j, 15, items)
                    emit_norm(att, acc, j, 1)
                for _ in gx:        # drain remaining proj-X steps
                    pass
                for _ in proj_gen(range(4, 8)):   # proj-Y
                    pass
        wpp.release()
        per.release()

    nc.compile()
    return nc


def _get_nc():
    if "nc" not in _cache:
        _cache["nc"] = _build()
    return _cache["nc"]


def _host_prep(x, Wqkv, bqkv, Wproj, bproj):
    x = np.ascontiguousarray(np.asarray(x, dtype=np.float32))
    Wqkv = np.asarray(Wqkv, dtype=np.float32)
    bqkv = np.asarray(bqkv, dtype=np.float32)
    Wproj = np.ascontiguousarray(np.asarray(Wproj, dtype=np.float32))
    bproj = np.asarray(bproj, dtype=np.float32)

    wq = np.ascontiguousarray(Wqkv[:, :C] * np.float32(0.125)).astype(NPBF)
    wk = np.ascontiguousarray(Wqkv[:, C:2*C]).astype(NPBF)
    wv = np.ascontiguousarray(Wqkv[:, 2*C:]).astype(NPBF)
    wp = Wproj.astype(NPBF)
    bq8 = (bqkv[:C] * np.float32(0.125)).reshape(8, 128, 1).copy()
    bk8 = bqkv[C:2*C].reshape(8, 128, 1).copy()
    bv = bqkv[2*C:]
    bpe = (bproj.astype(np.float64) + bv.astype(np.float64) @ Wproj.astype(np.float64)).astype(np.float32)
    bpeb = np.ascontiguousarray(np.broadcast_to(bpe, (128, C)))

    ridx = np.arange(128)[:, None]
    cidx = np.arange(128)[None, :]
    tri = (ridx <= cidx)
    masks = []
    for par in range(2):
        mk = np.zeros((128, 16 * 128), dtype=NPBF)
        for m in range(16):
            g = OWNT[par][m // 2]
            if m < g:
                mk[:, 128*m:128*(m+1)] = 1
            elif m == g:
                mk[:, 128*m:128*(m+1)] = tri
        masks.append(mk)

    in_maps = []
    for core in range(8):
        b, par = core // 2, core % 2
        xt = np.ascontiguousarray(x[b].T.astype(NPBF))
        xq = np.ascontiguousarray(
            np.concatenate([xt[:, 128*g:128*(g+1)] for g in OWNT[par]], axis=1))
        in_maps.append(dict(xt=xt, xq=xq, wq=wq, wk=wk, wv=wv, wp=wp,
                            bq=bq8, bk=bk8, bpeb=bpeb, masks=masks[par]))
    return in_maps


def kernel(x, Wqkv, bqkv, Wproj, bproj):
    nc = _get_nc()
    in_maps = _host_prep(x, Wqkv, bqkv, Wproj, bproj)
    trace = bool(os.environ.get("BASS_TRACE"))
    res = run_bass_kernel_spmd(nc, in_maps, list(range(8)), trace=trace)
    _cache["last_exec_time_ns"] = res.exec_time_ns
    _cache["last_res"] = res
    out = np.empty((B, T, C), dtype=np.float32)
    for core in range(8):
        b, par = core // 2, core % 2
        y = res.results[core]["y"]
        for si, g in enumerate(OWNT[par]):
            out[b, 128*g:128*(g+1)] = y[128*si:128*(si+1)]
    return out


# revision 12
# speedup vs baseline: 1.2508x; 1.0032x over previous
"""Causal self-attention on 8 TRN2 NeuronCores (Bass/Tile, SPMD).

Problem: B=4, T=2048, C=1024, H=16, D=64, fp32 in/out.

Sharding: core i = (batch b=i//2, parity p=i%2). Each core computes ALL 16
heads for 8 of the 16 128-wide query tiles of its batch: parity 0 owns
global q-tiles {0,3,4,7,8,11,12,15}, parity 1 owns {1,2,5,6,9,10,13,14}.
Sorted by causal extent, both parities pad to the SAME per-slot key-tile
extents E = [2,4,6,8,10,12,14,16] -> every core runs the IDENTICAL
instruction stream (SPMD); causality handled by host-supplied mask data
(mask m is triangular/ones/zeros depending on parity). K/V are computed
for the full sequence on both cores of a batch (duplication beats
communication here).

All tensor data is bf16 (PSUM accumulation fp32): same 1 cyc/row PE
stream rate as fp32r but half the SBUF/DMA traffic, half LDWEIGHTS cost,
4x DVE rate on the bf16 mask multiplies, and no N>=256 matmul
restriction (enables exact extents).

Per-core pipeline:
  1. K^T[d,t] (d on partitions, 2 heads/tile) via j-major matmuls with
     lhsT(weights) reused across t; V_aug[t,(h,d|1)] (ones column folded
     for softmax sums) reusing the same x tiles; Q^T likewise from the
     core's own (slot-sorted) query columns. PSUM evacuation (+bias) on
     the otherwise-idle Act engine.
  2. Attention per (head-pair j, pass, key-tile pair): the q-tiles
     needing key-tile m form a contiguous SUFFIX of the slot-sorted Q
     columns, so S^T(m) = K_m^T Q_suffix is ONE long matmul (N=512..128)
     per (half, m); batched exp on ScalarE over [128, 2N] PSUM spans;
     per-pair diagonal masks on DVE (bf16, 4x mode); PV accumulated into
     per-pass [65,512] PSUM accumulators (row 64 = softmax denominators),
     software-pipelined one pair behind S so PE never waits on exp.
     Two passes (slots 0-3, 4-7) keep PSUM within 8 banks.
     Normalize with DVE reciprocal + GpSimd broadcast; O^T stays in SBUF.
  3. Output projection Y = O^T.T @ Wproj + bias_eff directly from SBUF
     (bias_eff = bproj + bv @ Wproj, exact since softmax rows sum to 1).

Host: slices/transposes/casts inputs to bf16, precomputes masks +
effective bias, reassembles the 8 per-core [1024,1024] outputs.
"""
import os
import numpy as np
import ml_dtypes

import concourse.bacc as bacc
import concourse.mybir as mybir
import concourse.tile as tile
from concourse.bass_utils import run_bass_kernel_spmd

B, T, C, H, D = 4, 2048, 1024, 16, 64
F32 = mybir.dt.float32
BF16 = mybir.dt.bfloat16
NPBF = ml_dtypes.bfloat16
VA_W = H * (D + 1)            # 1040: V_aug cols = 16 heads x (64 + ones)
OWNT = [[0, 3, 4, 7, 8, 11, 12, 15], [1, 2, 5, 6, 9, 10, 13, 14]]
EXT = [2, 4, 6, 8, 10, 12, 14, 16]   # padded key-tile extent per slot

_cache = {}


def _build():
    nc = bacc.Bacc("TRN2", target_bir_lowering=False, debug=False,
                   enable_asserts=False, num_devices=8)
    def din(name, shape, dt=BF16):
        return nc.dram_tensor(name, list(shape), dt, kind="ExternalInput").ap()

    xt_d = din("xt", (C, T))            # x[b].T
    xq_d = din("xq", (C, 1024))         # own q columns of xt, slot-sorted
    wq_d = din("wq", (C, C))            # pre-scaled by 1/8
    wk_d = din("wk", (C, C))
    wv_d = din("wv", (C, C))
    wp_d = din("wp", (C, C))
    bq_d = din("bq", (8, 128, 1), F32)  # pre-scaled by 1/8
    bk_d = din("bk", (8, 128, 1), F32)
    bpeb_d = din("bpeb", (128, C), F32)  # bproj_eff broadcast to 128 partitions
    mk_d = din("masks", (128, 16 * 128))
    y_d = nc.dram_tensor("y", [1024, C], F32, kind="ExternalOutput").ap()

    bypass = mybir.AluOpType.bypass
    mult = mybir.AluOpType.mult
    add = mybir.AluOpType.add
    EXP = mybir.ActivationFunctionType.Exp

    with tile.TileContext(nc) as tc:
        # ---------------- persistent tiles ------------------------------
        per = tc.alloc_tile_pool(name="per", bufs=1)
        KT = [per.tile([128, T], BF16, name=f"kt{j}", tag=f"kt{j}") for j in range(8)]
        VA = [per.tile([128, VA_W], BF16, name=f"va{m}", tag=f"va{m}") for m in range(16)]
        QT = [per.tile([128, 1024], BF16, name=f"qt{j}", tag=f"qt{j}") for j in range(8)]
        OT = [per.tile([128, 1024], BF16, name=f"ot{j}", tag=f"ot{j}") for j in range(8)]
        MK = per.tile([128, 16 * 128], BF16, name="mk", tag="mk")
        bks = [per.tile([128, 1], F32, name=f"bks{j}", tag=f"bks{j}") for j in range(8)]
        bqs = [per.tile([128, 1], F32, name=f"bqs{j}", tag=f"bqs{j}") for j in range(8)]
        bpeb = per.tile([128, C], F32, name="bpeb", tag="bpeb")
        ones16 = per.tile([128, H], BF16, name="ones16", tag="ones16")

        # -------- phase 1: K^T, V_aug, Q^T ------------------------------
        wkvp = tc.alloc_tile_pool(name="wkvp", bufs=1)
        wkc = [wkvp.tile([128, C], BF16, name=f"wkc{c}", tag=f"wkc{c}") for c in range(8)]
        wvc = [wkvp.tile([128, C], BF16, name=f"wvc{c}", tag=f"wvc{c}") for c in range(8)]
        wqp = tc.alloc_tile_pool(name="wqp", bufs=1)
        wqc = [wqp.tile([128, C], BF16, name=f"wqc{c}", tag=f"wqc{c}") for c in range(8)]
        xqp = tc.alloc_tile_pool(name="xqp", bufs=1)
        xqc = [xqp.tile([128, 1024], BF16, name=f"xqc{c}", tag=f"xqc{c}") for c in range(8)]
        with tc.tile_pool(name="xsp", bufs=1) as xsp, \
             tc.tile_pool(name="ps1", bufs=3, space="PSUM") as ps1:
            # DMAs in priority order: K weights + first x slab first.
            for c in range(8):
                nc.sync.dma_start(out=wkc[c][:], in_=wk_d[128*c:128*(c+1), :])
            for j in range(8):
                nc.sync.dma_start(out=bks[j][:], in_=bk_d[j])
            xts0 = []
            for c in range(8):
                xt_t = xsp.tile([128, 1024], BF16, name=f"xts{c}", tag=f"xts{c}")
                nc.sync.dma_start(out=xt_t[:], in_=xt_d[128*c:128*(c+1), 0:1024])
                xts0.append(xt_t)
            for c in range(8):
                nc.sync.dma_start(out=wvc[c][:], in_=wv_d[128*c:128*(c+1), :])
            # prefetch: Q weights/inputs, masks, misc, then second x slab
            for c in range(8):
                nc.sync.dma_start(out=wqc[c][:], in_=wq_d[128*c:128*(c+1), :])
                nc.sync.dma_start(out=xqc[c][:], in_=xq_d[128*c:128*(c+1), :])
            for j in range(8):
                nc.sync.dma_start(out=bqs[j][:], in_=bq_d[j])
            nc.sync.dma_start(out=MK[:], in_=mk_d[:])
            nc.sync.dma_start(out=bpeb[:], in_=bpeb_d[:])
            xts1 = []
            for c in range(8):
                xt_t = xsp.tile([128, 1024], BF16, name=f"xts{c}", tag=f"xts{c}")
                nc.sync.dma_start(out=xt_t[:], in_=xt_d[128*c:128*(c+1), 1024:2048])
                xts1.append(xt_t)

            # ones columns of V_aug
            nc.vector.memset(ones16[:], 1.0)
            ones16_3d = ones16[:].unsqueeze(2)
            for m in range(16):
                dst1 = VA[m][:].rearrange("p (h d) -> p h d", d=D+1)[:, :, D:D+1]
                nc.vector.tensor_copy(out=dst1, in_=ones16_3d)

            for ss, xts in ((0, xts0), (1, xts1)):
                # K^T for this t-superslab: j-major, lhsT reused over 2 subs
                for j in range(8):
                    pk = ps1.tile([128, 1024], F32, name="pk", tag="pk")
                    for c in range(8):
                        for sub in range(2):
                            nc.tensor.matmul(out=pk[:, 512*sub:512*(sub+1)],
                                             lhsT=wkc[c][:, 128*j:128*(j+1)],
                                             rhs=xts[c][:, 512*sub:512*(sub+1)],
                                             start=(c == 0), stop=(c == 7))
                    nc.scalar.add(out=KT[j][:, 1024*ss:1024*(ss+1)], in_=pk[:],
                                  add=bks[j][:])
                # V for this superslab: 8 t-tiles (m = 8*ss + tt)
                for tt in range(8):
                    pv = ps1.tile([128, 1024], F32, name="pv", tag="pk")
                    for c in range(8):
                        for jc in range(2):
                            nc.tensor.matmul(out=pv[:, 512*jc:512*(jc+1)],
                                             lhsT=xts[c][:, 128*tt:128*(tt+1)],
                                             rhs=wvc[c][:, 512*jc:512*(jc+1)],
                                             start=(c == 0), stop=(c == 7))
                    dst = VA[8*ss+tt][:].rearrange("p (h d) -> p h d", d=D+1)[:, :, 0:D]
                    src = pv[:].rearrange("p (h d) -> p h d", d=D)
                    nc.scalar.copy(out=dst, in_=src)
            # Q^T (own columns, slot-sorted)
            for j in range(8):
                pq = ps1.tile([128, 1024], F32, name="pq", tag="pk")
                for c in range(8):
                    for sub in range(2):
                        nc.tensor.matmul(out=pq[:, 512*sub:512*(sub+1)],
                                         lhsT=wqc[c][:, 128*j:128*(j+1)],
                                         rhs=xqc[c][:, 512*sub:512*(sub+1)],
                                         start=(c == 0), stop=(c == 7))
                nc.scalar.add(out=QT[j][:, :], in_=pq[:], add=bqs[j][:])
        xqp.release()
        wqp.release()
        wkvp.release()

        # ---------------- phase 2: attention ----------------------------
        # X sweep (pass 0, slots 0-3) for all j, then Y sweep (pass 1) with
        # the X-half of the output projection interleaved, then proj-Y.
        wpp = tc.alloc_tile_pool(name="wpp", bufs=1)
        wpc = [wpp.tile([128, C], BF16, name=f"wpc{c}", tag=f"wpc{c}") for c in range(8)]

        def emit_pair_packed(ps, att, ss_bufs, j, p, g):
            """All 4 streams (2 halves x 2 key-tiles) in one ss tile, one exp.
            Only valid when 4N <= 1024 (no PSUM bank crossing possible)."""
            m0, m1 = 2*g, 2*g + 1
            so = max(0, g - 4*p)
            qc0 = 512*p + 128*so
            N = 512 - 128*so
            masked = (g >= 4*p)
            halves = [(0, [(0, m0), (1, m1)]), (1, [(2, m0), (3, m1)])]
            ss_t = ps.tile([128, 1024], F32, name="ss", tag="ss", bufs=ss_bufs)
            for half, ums in halves:
                r0, r1 = 64*half, 64*(half+1)
                for u, m in ums:
                    nc.tensor.matmul(out=ss_t[:, N*u:N*(u+1)],
                                     lhsT=KT[j][r0:r1, 128*m:128*(m+1)],
                                     rhs=QT[j][r0:r1, qc0:qc0+N],
                                     tile_position=(r0, 0),
                                     start=True, stop=True)
            pt = att.tile([128, 1024], BF16, name="pt", tag="pt", bufs=8)
            nc.scalar.activation(out=pt[:, 0:4*N], in_=ss_t[:, 0:4*N], func=EXP)
            out = []
            for half, ums in halves:
                if masked:
                    for u, m in ums:
                        nc.vector.scalar_tensor_tensor(
                            out=pt[:, N*u:N*u+128], in0=pt[:, N*u:N*u+128],
                            scalar=0.0, in1=MK[:, 128*m:128*(m+1)],
                            op0=bypass, op1=mult)
                out.append((pt, N, so, ums, half))
            return out

        def emit_pair_unpacked(ps, att, ss_bufs, j, p, g):
            m0, m1 = 2*g, 2*g + 1
            so = max(0, g - 4*p)
            qc0 = 512*p + 128*so
            N = 512 - 128*so
            masked = (g >= 4*p)
            out = []
            for half in range(2):
                r0, r1 = 64*half, 64*(half+1)
                ss_t = ps.tile([128, 1024], F32, name="ss", tag="ss", bufs=ss_bufs)
                for u, m in ((0, m0), (1, m1)):
                    c0 = N * u
                    while c0 < N * (u + 1):
                        c1 = min(N * (u + 1), (c0 // 512 + 1) * 512)
                        nc.tensor.matmul(out=ss_t[:, c0:c1],
                                         lhsT=KT[j][r0:r1, 128*m:128*(m+1)],
                                         rhs=QT[j][r0:r1, qc0 + c0 - N*u:
                                                          qc0 + c1 - N*u],
                                         tile_position=(r0, 0),
                                         start=True, stop=True)
                        c0 = c1
                pt = att.tile([128, 1024], BF16, name="pt", tag="pt", bufs=8)
                nc.scalar.activation(out=pt[:, 0:2*N], in_=ss_t[:, 0:2*N], func=EXP)
                if masked:
                    for u, m in ((0, m0), (1, m1)):
                        nc.vector.scalar_tensor_tensor(
                            out=pt[:, N*u:N*u+128], in0=pt[:, N*u:N*u+128],
                            scalar=0.0, in1=MK[:, 128*m:128*(m+1)],
                            op0=bypass, op1=mult)
                out.append((pt, N, so, [(0, m0), (1, m1)], half))
            return out

        def emit_attn_pair(ps, att, ss_bufs, j, p, g):
            so = max(0, g - 4*p)
            N = 512 - 128*so
            if 4*N == 1024:     # packed halves land in separate PSUM banks
                return emit_pair_packed(ps, att, ss_bufs, j, p, g)
            return emit_pair_unpacked(ps, att, ss_bufs, j, p, g)

        def emit_pv(acc, j, mlast, items):
            for (ppt, pN, pso, ums, phalf) in items:
                hh = 2*j + phalf
                for u, m in ums:
                    nc.tensor.matmul(out=acc[phalf][:, 128*pso:512],
                                     lhsT=VA[m][:, 65*hh:65*(hh+1)],
                                     rhs=ppt[:, pN*u:pN*(u+1)],
                                     start=(m == 0), stop=(m == mlast),
                                     skip_group_check=True)

        def emit_norm(att, acc, j, p):
            for half in range(2):
                lsb = att.tile([1, 512], F32, name="lsb", tag=f"lsb{half}", bufs=2)
                nc.vector.tensor_copy(out=lsb[:], in_=acc[half][64:65, :])
                rsb = att.tile([1, 512], F32, name="rsb", tag=f"rsb{half}", bufs=2)
                nc.vector.reciprocal_approx_fast(rsb[:], lsb[:])
                rbb = att.tile([64, 512], F32, name="rbb", tag=f"rbb{half}", bufs=2)
                nc.gpsimd.partition_broadcast(rbb[:], rsb[:])
                nc.vector.scalar_tensor_tensor(
                    out=OT[j][64*half:64*(half+1), 512*p:512*(p+1)],
                    in0=acc[half][0:64, :], scalar=0.0, in1=rbb[:],
                    op0=bypass, op1=mult)

        LAG = 2
        with tc.tile_pool(name="att", bufs=1) as att:
            for c in range(8):
                nc.sync.dma_start(out=wpc[c][:], in_=wp_d[128*c:128*(c+1), :])
            # ---- X sweep: pass 0 for all j (deep ss pipeline) ----
            with tc.tile_pool(name="psx", bufs=1, space="PSUM") as psx:
                for j in range(8):
                    acc = [psx.tile([65, 512], F32, name=f"acc{h}", tag=f"acc{h}")
                           for h in range(2)]
                    pend = []
                    for g in range(4):
                        pend.append(emit_attn_pair(psx, att, 3, j, 0, g))
                        if len(pend) > LAG:
                            emit_pv(acc, j, 7, pend.pop(0))
                    for items in pend:
                        emit_pv(acc, j, 7, items)
                    emit_norm(att, acc, j, 0)
            # ---- Y sweep: pass 1 for all j, proj-X interleaved ----
            with tc.tile_pool(name="psy", bufs=1, space="PSUM") as psy:
                def proj_gen(tis):
                    for ti in tis:
                        py = [psy.tile([128, 512], F32, name=f"py{jc}",
                                       tag=f"py{jc}", bufs=1) for jc in range(2)]
                        for ci in range(8):
                            for jc in range(2):
                                nc.tensor.matmul(out=py[jc][:],
                                                 lhsT=OT[ci][:, 128*ti:128*(ti+1)],
                                                 rhs=wpc[ci][:, 512*jc:512*(jc+1)],
                                                 start=(ci == 0), stop=(ci == 7))
                            yield
                        for jc in range(2):
                            ysb = att.tile([128, 512], F32, name="ysb", tag="ysb",
                                           bufs=3)
                            nc.vector.scalar_tensor_tensor(
                                out=ysb[:], in0=py[jc][:], scalar=0.0,
                                in1=bpeb[:, 512*jc:512*(jc+1)],
                                op0=bypass, op1=add)
                            nc.sync.dma_start(
                                out=y_d[128*ti:128*(ti+1), 512*jc:512*(jc+1)],
                                in_=ysb[:])
                        yield
                gx = proj_gen(range(4))
                for j in range(8):
                    acc = [psy.tile([65, 512], F32, name=f"acc{h}", tag=f"acc{h}")
                           for h in range(2)]
                    pend = []
                    for g in range(8):
                        pend.append(emit_attn_pair(psy, att, 2, j, 1, g))
                        next(gx, None)
                        if len(pend) > LAG:
                            emit_pv(acc, j, 15, pend.pop(0))
                    for items in pend:
                        emit_pv(acc, j, 15, items)
                    emit_norm(att, acc, j, 1)
                for _ in gx:        # drain remaining proj-X steps
                    pass
                for _ in proj_gen(range(4, 8)):   # proj-Y
                    pass
        wpp.release()
        per.release()

    nc.compile()
    return nc


def _get_nc():
    if "nc" not in _cache:
        _cache["nc"] = _build()
    return _cache["nc"]


def _host_prep(x, Wqkv, bqkv, Wproj, bproj):
    x = np.ascontiguousarray(np.asarray(x, dtype=np.float32))
    Wqkv = np.asarray(Wqkv, dtype=np.float32)
    bqkv = np.asarray(bqkv, dtype=np.float32)
    Wproj = np.ascontiguousarray(np.asarray(Wproj, dtype=np.float32))
    bproj = np.asarray(bproj, dtype=np.float32)

    wq = np.ascontiguousarray(Wqkv[:, :C] * np.float32(0.125)).astype(NPBF)
    wk = np.ascontiguousarray(Wqkv[:, C:2*C]).astype(NPBF)
    wv = np.ascontiguousarray(Wqkv[:, 2*C:]).astype(NPBF)
    wp = Wproj.astype(NPBF)
    bq8 = (bqkv[:C] * np.float32(0.125)).reshape(8, 128, 1).copy()
    bk8 = bqkv[C:2*C].reshape(8, 128, 1).copy()
    bv = bqkv[2*C:]
    bpe = (bproj.astype(np.float64) + bv.astype(np.float64) @ Wproj.astype(np.float64)).astype(np.float32)
    bpeb = np.ascontiguousarray(np.broadcast_to(bpe, (128, C)))

    ridx = np.arange(128)[:, None]
    cidx = np.arange(128)[None, :]
    tri = (ridx <= cidx)
    masks = []
    for par in range(2):
        mk = np.zeros((128, 16 * 128), dtype=NPBF)
        for m in range(16):
            g = OWNT[par][m // 2]
            if m < g:
                mk[:, 128*m:128*(m+1)] = 1
            elif m == g:
                mk[:, 128*m:128*(m+1)] = tri
        masks.append(mk)

    in_maps = []
    for core in range(8):
        b, par = core // 2, core % 2
        xt = np.ascontiguousarray(x[b].T.astype(NPBF))
        xq = np.ascontiguousarray(
            np.concatenate([xt[:, 128*g:128*(g+1)] for g in OWNT[par]], axis=1))
        in_maps.append(dict(xt=xt, xq=xq, wq=wq, wk=wk, wv=wv, wp=wp,
                            bq=bq8, bk=bk8, bpeb=bpeb, masks=masks[par]))
    return in_maps


def kernel(x, Wqkv, bqkv, Wproj, bproj):
    nc = _get_nc()
    in_maps = _host_prep(x, Wqkv, bqkv, Wproj, bproj)
    trace = bool(os.environ.get("BASS_TRACE"))
    res = run_bass_kernel_spmd(nc, in_maps, list(range(8)), trace=trace)
    _cache["last_exec_time_ns"] = res.exec_time_ns
    _cache["last_res"] = res
    out = np.empty((B, T, C), dtype=np.float32)
    for core in range(8):
        b, par = core // 2, core % 2
        y = res.results[core]["y"]
        for si, g in enumerate(OWNT[par]):
            out[b, 128*g:128*(g+1)] = y[128*si:128*(si+1)]
    return out


# revision 15
# speedup vs baseline: 1.4858x; 1.1879x over previous
"""Causal self-attention on 8 TRN2 NeuronCores (Bass/Tile, SPMD).

Problem: B=4, T=2048, C=1024, H=16, D=64, fp32 in/out.

Sharding: core i = (batch b=i//2, parity p=i%2). Each core computes ALL 16
heads for 8 of the 16 128-wide query tiles of its batch: parity 0 owns
global q-tiles {0,3,4,7,8,11,12,15}, parity 1 owns {1,2,5,6,9,10,13,14}.
Sorted by causal extent, both parities pad to the SAME per-slot key-tile
extents E = [2,4,6,8,10,12,14,16] -> every core runs the IDENTICAL
instruction stream (SPMD); causality handled by host-supplied mask data
(mask m is triangular/ones/zeros depending on parity). K/V are computed
for the full sequence on both cores of a batch.

All tensor data is bf16 (PSUM accumulation fp32). Schedule is built to
keep the PE continuously busy (it only reaches its 2.4 GHz p-state after
~3us of uninterrupted work):
  part 1: K^T/V for t-superslab 0 + all of Q^T (j-major matmuls, weights
          stationary; PSUM evacuated with +bias on the Act engine).
  X sweep: attention pass 0 (slots 0-3, key tiles < 8) for all j,
          software-pipelined (PV lags S/exp by 2 pairs), with superslab-1
          K^T/V matmuls interleaved as filler so exp latency never idles
          the PE.
  Y sweep: attention pass 1 (slots 4-7, all 16 key tiles), two j's
          interleaved (independent chains) as mutual filler.
  tail:   output projection Y = O^T.T @ Wproj + bias_eff from SBUF
          (bias_eff = bproj + bv @ Wproj, exact since softmax rows sum
          to 1).

Attention inner scheme: q-tiles needing key-tile m form a contiguous
suffix of the slot-sorted Q columns, so S^T(m) = K_m^T Q_suffix is ONE
long matmul (N=512..128) per (half, m); matmul PSUM outputs are split at
512-col PSUM bank boundaries (HW corrupts writes that cross a bank);
batched exp on ScalarE; per-pair diagonal masks on DVE; PV accumulates
into per-pass [65,512] PSUM accumulators (row 64 = softmax denominator);
normalize via DVE reciprocal + GpSimd partition broadcast; O^T in SBUF.

Host: slices/transposes/casts inputs to bf16, precomputes masks +
effective bias, reassembles the 8 per-core [1024,1024] outputs.
"""
import os
import numpy as np
import ml_dtypes

import concourse.bacc as bacc
import concourse.mybir as mybir
import concourse.tile as tile
from concourse.bass_utils import run_bass_kernel_spmd

B, T, C, H, D = 4, 2048, 1024, 16, 64
F32 = mybir.dt.float32
BF16 = mybir.dt.bfloat16
NPBF = ml_dtypes.bfloat16
VA_W = H * (D + 1)            # 1040: V_aug cols = 16 heads x (64 + ones)
OWNT = [[0, 3, 4, 7, 8, 11, 12, 15], [1, 2, 5, 6, 9, 10, 13, 14]]
EXT = [2, 4, 6, 8, 10, 12, 14, 16]   # padded key-tile extent per slot

_cache = {}


def _build():
    nc = bacc.Bacc("TRN2", target_bir_lowering=False, debug=False,
                   enable_asserts=False, num_devices=8)
    def din(name, shape, dt=BF16):
        return nc.dram_tensor(name, list(shape), dt, kind="ExternalInput").ap()

    xt_d = din("xt", (C, T))            # x[b].T
    xq_d = din("xq", (C, 1024))         # own q columns of xt, slot-sorted
    wq_d = din("wq", (C, C))            # pre-scaled by 1/8
    wk_d = din("wk", (C, C))
    wv_d = din("wv", (C, C))
    wp_d = din("wp", (C, C))
    bq_d = din("bq", (8, 128, 1), F32)  # pre-scaled by 1/8
    bk_d = din("bk", (8, 128, 1), F32)
    bpeb_d = din("bpeb", (128, C), F32)
    mk_d = din("masks", (128, 16 * 128))
    y_d = nc.dram_tensor("y", [1024, C], F32, kind="ExternalOutput").ap()

    bypass = mybir.AluOpType.bypass
    mult = mybir.AluOpType.mult
    add = mybir.AluOpType.add
    EXP = mybir.ActivationFunctionType.Exp

    with tile.TileContext(nc) as tc:
        # ---------------- persistent tiles ------------------------------
        per = tc.alloc_tile_pool(name="per", bufs=1)
        KT = [per.tile([128, T], BF16, name=f"kt{j}", tag=f"kt{j}") for j in range(8)]
        VA = [per.tile([128, VA_W], BF16, name=f"va{m}", tag=f"va{m}") for m in range(16)]
        QT = [per.tile([128, 1024], BF16, name=f"qt{j}", tag=f"qt{j}") for j in range(8)]
        OT = [per.tile([128, 1024], BF16, name=f"ot{j}", tag=f"ot{j}") for j in range(8)]
        MK = per.tile([128, 16 * 128], BF16, name="mk", tag="mk")
        bks = [per.tile([128, 1], F32, name=f"bks{j}", tag=f"bks{j}") for j in range(8)]
        bqs = [per.tile([128, 1], F32, name=f"bqs{j}", tag=f"bqs{j}") for j in range(8)]
        bpeb = per.tile([128, C], F32, name="bpeb", tag="bpeb")
        ones16 = per.tile([128, H], BF16, name="ones16", tag="ones16")

        wkvp = tc.alloc_tile_pool(name="wkvp", bufs=1)
        wkc = [wkvp.tile([128, C], BF16, name=f"wkc{c}", tag=f"wkc{c}") for c in range(8)]
        wvc = [wkvp.tile([128, C], BF16, name=f"wvc{c}", tag=f"wvc{c}") for c in range(8)]
        wpp = tc.alloc_tile_pool(name="wpp", bufs=1)
        wpc = [wpp.tile([128, C], BF16, name=f"wpc{c}", tag=f"wpc{c}") for c in range(8)]
        xs1p = tc.alloc_tile_pool(name="xs1p", bufs=1)
        xts1 = [xs1p.tile([128, 1024], BF16, name=f"x1_{c}", tag=f"x1_{c}")
                for c in range(8)]

        def k_block(ps1, xts, j, ss):
            """K^T block j for one t-superslab (16 matmuls + Act evac)."""
            pk = ps1.tile([128, 1024], F32, name="pk", tag="pk")
            for c in range(8):
                for sub in range(2):
                    nc.tensor.matmul(out=pk[:, 512*sub:512*(sub+1)],
                                     lhsT=wkc[c][:, 128*j:128*(j+1)],
                                     rhs=xts[c][:, 512*sub:512*(sub+1)],
                                     start=(c == 0), stop=(c == 7))
            nc.scalar.add(out=KT[j][:, 1024*ss:1024*(ss+1)], in_=pk[:],
                          add=bks[j][:])

        def v_block(ps1, xts, tt, ss):
            """V_aug t-tile m=8*ss+tt (16 matmuls + Act evac, rearranged)."""
            pv = ps1.tile([128, 1024], F32, name="pv", tag="pk")
            for c in range(8):
                for jc in range(2):
                    nc.tensor.matmul(out=pv[:, 512*jc:512*(jc+1)],
                                     lhsT=xts[c][:, 128*tt:128*(tt+1)],
                                     rhs=wvc[c][:, 512*jc:512*(jc+1)],
                                     start=(c == 0), stop=(c == 7))
            dst = VA[8*ss+tt][:].rearrange("p (h d) -> p h d", d=D+1)[:, :, 0:D]
            src = pv[:].rearrange("p (h d) -> p h d", d=D)
            nc.scalar.copy(out=dst, in_=src)

        # -------- part 1: superslab-0 K^T and V, all of Q^T -------------
        with tc.tile_pool(name="wqp", bufs=1) as wqp, \
             tc.tile_pool(name="xs0p", bufs=1) as xs0p, \
             tc.tile_pool(name="ps1", bufs=3, space="PSUM") as ps1:
            wqc = [wqp.tile([128, C], BF16, name=f"wqc{c}", tag=f"wqc{c}")
                   for c in range(8)]
            # priority DMAs: interleave K weights with x superslab 0
            xts0 = []
            for c in range(8):
                nc.sync.dma_start(out=wkc[c][:], in_=wk_d[128*c:128*(c+1), :])
                xt_t = xs0p.tile([128, 1024], BF16, name=f"x0_{c}", tag=f"x0_{c}")
                nc.sync.dma_start(out=xt_t[:], in_=xt_d[128*c:128*(c+1), 0:1024])
                xts0.append(xt_t)
            for j in range(8):
                nc.sync.dma_start(out=bks[j][:], in_=bk_d[j])
            for c in range(8):
                nc.sync.dma_start(out=wvc[c][:], in_=wv_d[128*c:128*(c+1), :])
            for c in range(8):
                nc.sync.dma_start(out=wqc[c][:], in_=wq_d[128*c:128*(c+1), :])
            for j in range(8):
                nc.sync.dma_start(out=bqs[j][:], in_=bq_d[j])
            nc.sync.dma_start(out=MK[:], in_=mk_d[:])
            nc.sync.dma_start(out=bpeb[:], in_=bpeb_d[:])
            for c in range(8):
                nc.sync.dma_start(out=xts1[c][:], in_=xt_d[128*c:128*(c+1), 1024:2048])
            # xq reuses the xts0 buffers (second tile per tag)
            xqc = []
            for c in range(8):
                xq_t = xs0p.tile([128, 1024], BF16, name=f"x0_{c}", tag=f"x0_{c}")
                nc.sync.dma_start(out=xq_t[:], in_=xq_d[128*c:128*(c+1), :])
                xqc.append(xq_t)

            nc.vector.memset(ones16[:], 1.0)
            ones16_3d = ones16[:].unsqueeze(2)
            for m in range(16):
                dst1 = VA[m][:].rearrange("p (h d) -> p h d", d=D+1)[:, :, D:D+1]
                nc.vector.tensor_copy(out=dst1, in_=ones16_3d)

            for j in range(8):
                k_block(ps1, xts0, j, 0)
            for tt in range(8):
                v_block(ps1, xts0, tt, 0)
            for j in range(8):
                pq = ps1.tile([128, 1024], F32, name="pq", tag="pk")
                for c in range(8):
                    for sub in range(2):
                        nc.tensor.matmul(out=pq[:, 512*sub:512*(sub+1)],
                                         lhsT=wqc[c][:, 128*j:128*(j+1)],
                                         rhs=xqc[c][:, 512*sub:512*(sub+1)],
                                         start=(c == 0), stop=(c == 7))
                nc.scalar.add(out=QT[j][:, :], in_=pq[:], add=bqs[j][:])

        # ---------------- attention helpers -----------------------------
        def emit_pair_packed(ps, att, ss_bufs, j, p, g):
            m0, m1 = 2*g, 2*g + 1
            so = max(0, g - 4*p)
            qc0 = 512*p + 128*so
            N = 512 - 128*so
            masked = (g >= 4*p)
            halves = [(0, [(0, m0), (1, m1)]), (1, [(2, m0), (3, m1)])]
            ss_t = ps.tile([128, 1024], F32, name="ss", tag="ss", bufs=ss_bufs)
            for half, ums in halves:
                r0, r1 = 64*half, 64*(half+1)
                for u, m in ums:
                    nc.tensor.matmul(out=ss_t[:, N*u:N*(u+1)],
                                     lhsT=KT[j][r0:r1, 128*m:128*(m+1)],
                                     rhs=QT[j][r0:r1, qc0:qc0+N],
                                     tile_position=(r0, 0),
                                     start=True, stop=True)
            pt = att.tile([128, 1024], BF16, name="pt", tag="pt", bufs=10)
            nc.scalar.activation(out=pt[:, 0:4*N], in_=ss_t[:, 0:4*N], func=EXP)
            out = []
            for half, ums in halves:
                if masked:
                    for u, m in ums:
                        nc.vector.scalar_tensor_tensor(
                            out=pt[:, N*u:N*u+128], in0=pt[:, N*u:N*u+128],
                            scalar=0.0, in1=MK[:, 128*m:128*(m+1)],
                            op0=bypass, op1=mult)
                out.append((pt, N, so, ums, half))
            return out

        def emit_pair_unpacked(ps, att, ss_bufs, j, p, g):
            m0, m1 = 2*g, 2*g + 1
            so = max(0, g - 4*p)
            qc0 = 512*p + 128*so
            N = 512 - 128*so
            masked = (g >= 4*p)
            out = []
            for half in range(2):
                r0, r1 = 64*half, 64*(half+1)
                ss_t = ps.tile([128, 1024], F32, name="ss", tag="ss", bufs=ss_bufs)
                for u, m in ((0, m0), (1, m1)):
                    c0 = N * u           # split dst at PSUM bank boundaries
                    while c0 < N * (u + 1):
                        c1 = min(N * (u + 1), (c0 // 512 + 1) * 512)
                        nc.tensor.matmul(out=ss_t[:, c0:c1],
                                         lhsT=KT[j][r0:r1, 128*m:128*(m+1)],
                                         rhs=QT[j][r0:r1, qc0 + c0 - N*u:
                                                          qc0 + c1 - N*u],
                                         tile_position=(r0, 0),
                                         start=True, stop=True)
                        c0 = c1
                pt = att.tile([128, 1024], BF16, name="pt", tag="pt", bufs=10)
                nc.scalar.activation(out=pt[:, 0:2*N], in_=ss_t[:, 0:2*N], func=EXP)
                if masked:
                    for u, m in ((0, m0), (1, m1)):
                        nc.vector.scalar_tensor_tensor(
                            out=pt[:, N*u:N*u+128], in0=pt[:, N*u:N*u+128],
                            scalar=0.0, in1=MK[:, 128*m:128*(m+1)],
                            op0=bypass, op1=mult)
                out.append((pt, N, so, [(0, m0), (1, m1)], half))
            return out

        def emit_attn_pair(ps, att, ss_bufs, j, p, g):
            so = max(0, g - 4*p)
            N = 512 - 128*so
            if 4*N == 1024:     # packed halves land in separate PSUM banks
                return emit_pair_packed(ps, att, ss_bufs, j, p, g)
            return emit_pair_unpacked(ps, att, ss_bufs, j, p, g)

        def emit_pv(acc, j, mlast, items):
            for (ppt, pN, pso, ums, phalf) in items:
                hh = 2*j + phalf
                for u, m in ums:
                    nc.tensor.matmul(out=acc[phalf][:, 128*pso:512],
                                     lhsT=VA[m][:, 65*hh:65*(hh+1)],
                                     rhs=ppt[:, pN*u:pN*(u+1)],
                                     start=(m == 0), stop=(m == mlast),
                                     skip_group_check=True)

        def emit_norm(att, acc, j, p):
            for half in range(2):
                lsb = att.tile([1, 512], F32, name="lsb", tag=f"lsb{half}", bufs=1)
                nc.vector.tensor_copy(out=lsb[:], in_=acc[half][64:65, :])
                rsb = att.tile([1, 512], F32, name="rsb", tag=f"rsb{half}", bufs=1)
                nc.vector.reciprocal_approx_fast(rsb[:], lsb[:])
                rbb = att.tile([64, 512], F32, name="rbb", tag=f"rbb{half}", bufs=1)
                nc.gpsimd.partition_broadcast(rbb[:], rsb[:])
                nc.vector.scalar_tensor_tensor(
                    out=OT[j][64*half:64*(half+1), 512*p:512*(p+1)],
                    in0=acc[half][0:64, :], scalar=0.0, in1=rbb[:],
                    op0=bypass, op1=mult)

        LAG = 2
        # ---- X sweep: pass 0 for all j; superslab-1 K/V as PE filler ----
        with tc.tile_pool(name="attx", bufs=1) as attx, \
             tc.tile_pool(name="psx", bufs=1, space="PSUM") as psx:
            for c in range(8):
                nc.sync.dma_start(out=wpc[c][:], in_=wp_d[128*c:128*(c+1), :])
            filler = []
            for j in range(8):
                filler.append(("k", j))
            for tt in range(8):
                filler.append(("v", tt))
            fi = 0
            for j in range(8):
                acc = [psx.tile([65, 512], F32, name=f"acc{h}", tag=f"acc{h}")
                       for h in range(2)]
                pend = []
                for g in range(4):
                    pend.append(emit_attn_pair(psx, attx, 2, j, 0, g))
                    if fi < len(filler):   # one filler block per pair
                        kind, idx = filler[fi]; fi += 1
                        if kind == "k":
                            k_block(psx, xts1, idx, 1)
                        else:
                            v_block(psx, xts1, idx, 1)
                    if len(pend) > LAG:
                        emit_pv(acc, j, 7, pend.pop(0))
                for items in pend:
                    emit_pv(acc, j, 7, items)
                emit_norm(attx, acc, j, 0)
            while fi < len(filler):
                kind, idx = filler[fi]; fi += 1
                if kind == "k":
                    k_block(psx, xts1, idx, 1)
                else:
                    v_block(psx, xts1, idx, 1)
        xs1p.release()

        # ---- Y sweep: pass 1, two j's interleaved ----------------------
        with tc.tile_pool(name="atty", bufs=1) as atty:
            with tc.tile_pool(name="psy", bufs=1, space="PSUM") as psy:
                for jp in range(4):
                    js = (2*jp, 2*jp + 1)
                    acc = {jj: [psy.tile([65, 512], F32, name=f"acc{jj%2}{h}",
                                         tag=f"acc{jj%2}{h}") for h in range(2)]
                           for jj in js}
                    pend = []
                    for g in range(8):
                        for jj in js:
                            pend.append((jj, emit_attn_pair(psy, atty, 2, jj, 1, g)))
                            if len(pend) > 2*LAG - 1:
                                pj, items = pend.pop(0)
                                emit_pv(acc[pj], pj, 15, items)
                    for pj, items in pend:
                        emit_pv(acc[pj], pj, 15, items)
                    for jj in js:
                        emit_norm(atty, acc[jj], jj, 1)
            # ---- projection tail (all 8 q-tiles) ----
            with tc.tile_pool(name="psp", bufs=1, space="PSUM") as psp:
                for ti in range(8):
                    py = [psp.tile([128, 512], F32, name=f"py{jc}",
                                   tag=f"py{jc}", bufs=2) for jc in range(2)]
                    for ci in range(8):
                        for jc in range(2):
                            nc.tensor.matmul(out=py[jc][:],
                                             lhsT=OT[ci][:, 128*ti:128*(ti+1)],
                                             rhs=wpc[ci][:, 512*jc:512*(jc+1)],
                                             start=(ci == 0), stop=(ci == 7))
                    for jc in range(2):
                        ysb = atty.tile([128, 512], F32, name="ysb", tag="ysb",
                                        bufs=3)
                        nc.vector.scalar_tensor_tensor(
                            out=ysb[:], in0=py[jc][:], scalar=0.0,
                            in1=bpeb[:, 512*jc:512*(jc+1)],
                            op0=bypass, op1=add)
                        nc.sync.dma_start(
                            out=y_d[128*ti:128*(ti+1), 512*jc:512*(jc+1)],
                            in_=ysb[:])
        wpp.release()
        wkvp.release()
        per.release()

    nc.compile()
    return nc


def _get_nc():
    if "nc" not in _cache:
        _cache["nc"] = _build()
    return _cache["nc"]


def _host_prep(x, Wqkv, bqkv, Wproj, bproj):
    x = np.ascontiguousarray(np.asarray(x, dtype=np.float32))
    Wqkv = np.asarray(Wqkv, dtype=np.float32)
    bqkv = np.asarray(bqkv, dtype=np.float32)
    Wproj = np.ascontiguousarray(np.asarray(Wproj, dtype=np.float32))
    bproj = np.asarray(bproj, dtype=np.float32)

    wq = np.ascontiguousarray(Wqkv[:, :C] * np.float32(0.125)).astype(NPBF)
    wk = np.ascontiguousarray(Wqkv[:, C:2*C]).astype(NPBF)
    wv = np.ascontiguousarray(Wqkv[:, 2*C:]).astype(NPBF)
    wp = Wproj.astype(NPBF)
    bq8 = (bqkv[:C] * np.float32(0.125)).reshape(8, 128, 1).copy()
    bk8 = bqkv[C:2*C].reshape(8, 128, 1).copy()
    bv = bqkv[2*C:]
    bpe = (bproj.astype(np.float64) + bv.astype(np.float64) @ Wproj.astype(np.float64)).astype(np.float32)
    bpeb = np.ascontiguousarray(np.broadcast_to(bpe, (128, C)))

    ridx = np.arange(128)[:, None]
    cidx = np.arange(128)[None, :]
    tri = (ridx <= cidx)
    masks = []
    for par in range(2):
        mk = np.zeros((128, 16 * 128), dtype=NPBF)
        for m in range(16):
            g = OWNT[par][m // 2]
            if m < g:
                mk[:, 128*m:128*(m+1)] = 1
            elif m == g:
                mk[:, 128*m:128*(m+1)] = tri
        masks.append(mk)

    in_maps = []
    for core in range(8):
        b, par = core // 2, core % 2
        xt = np.ascontiguousarray(x[b].T.astype(NPBF))
        xq = np.ascontiguousarray(
            np.concatenate([xt[:, 128*g:128*(g+1)] for g in OWNT[par]], axis=1))
        in_maps.append(dict(xt=xt, xq=xq, wq=wq, wk=wk, wv=wv, wp=wp,
                            bq=bq8, bk=bk8, bpeb=bpeb, masks=masks[par]))
    return in_maps


def kernel(x, Wqkv, bqkv, Wproj, bproj):
    nc = _get_nc()
    in_maps = _host_prep(x, Wqkv, bqkv, Wproj, bproj)
    trace = bool(os.environ.get("BASS_TRACE"))
    res = run_bass_kernel_spmd(nc, in_maps, list(range(8)), trace=trace)
    _cache["last_exec_time_ns"] = res.exec_time_ns
    _cache["last_res"] = res
    out = np.empty((B, T, C), dtype=np.float32)
    for core in range(8):
        b, par = core // 2, core % 2
        y = res.results[core]["y"]
        for si, g in enumerate(OWNT[par]):
            out[b, 128*g:128*(g+1)] = y[128*si:128*(si+1)]
    return out


# revision 19
# speedup vs baseline: 1.4942x; 1.0057x over previous
"""Causal self-attention on 8 TRN2 NeuronCores (Bass/Tile, SPMD).

Problem: B=4, T=2048, C=1024, H=16, D=64, fp32 in/out.

Sharding: core i = (batch b=i//2, parity p=i%2). Each core computes ALL 16
heads for 8 of the 16 128-wide query tiles of its batch: parity 0 owns
global q-tiles {0,3,4,7,8,11,12,15}, parity 1 owns {1,2,5,6,9,10,13,14}.
Sorted by causal extent, both parities pad to the SAME per-slot key-tile
extents E = [2,4,6,8,10,12,14,16] -> every core runs the IDENTICAL
instruction stream (SPMD); causality handled by host-supplied mask data
(mask m is triangular/ones/zeros depending on parity). K/V are computed
for the full sequence on both cores of a batch.

All tensor data is bf16 (PSUM accumulation fp32). Schedule is built to
keep the PE continuously busy (it only reaches its 2.4 GHz p-state after
~3us of uninterrupted work):
  part 1: K^T/V for t-superslab 0 + all of Q^T (j-major matmuls, weights
          stationary; PSUM evacuated with +bias on the Act engine).
  X sweep: attention pass 0 (slots 0-3, key tiles < 8) for all j,
          software-pipelined (PV lags S/exp by 2 pairs), with superslab-1
          K^T/V matmuls interleaved as filler so exp latency never idles
          the PE.
  Y sweep: attention pass 1 (slots 4-7, all 16 key tiles), two j's
          interleaved (independent chains) as mutual filler.
  tail:   output projection Y = O^T.T @ Wproj + bias_eff from SBUF
          (bias_eff = bproj + bv @ Wproj, exact since softmax rows sum
          to 1).

Attention inner scheme: q-tiles needing key-tile m form a contiguous
suffix of the slot-sorted Q columns, so S^T(m) = K_m^T Q_suffix is ONE
long matmul (N=512..128) per (half, m); matmul PSUM outputs are split at
512-col PSUM bank boundaries (HW corrupts writes that cross a bank);
batched exp on ScalarE; per-pair diagonal masks on DVE; PV accumulates
into per-pass [65,512] PSUM accumulators (row 64 = softmax denominator);
normalize via DVE reciprocal + GpSimd partition broadcast; O^T in SBUF.

Host: slices/transposes/casts inputs to bf16, precomputes masks +
effective bias, reassembles the 8 per-core [1024,1024] outputs.
"""
import os
import numpy as np
import ml_dtypes

import concourse.bacc as bacc
import concourse.mybir as mybir
import concourse.tile as tile
from concourse.bass_utils import run_bass_kernel_spmd

B, T, C, H, D = 4, 2048, 1024, 16, 64
F32 = mybir.dt.float32
BF16 = mybir.dt.bfloat16
NPBF = ml_dtypes.bfloat16
VA_W = H * (D + 1)            # 1040: V_aug cols = 16 heads x (64 + ones)
OWNT = [[0, 3, 4, 7, 8, 11, 12, 15], [1, 2, 5, 6, 9, 10, 13, 14]]
EXT = [2, 4, 6, 8, 10, 12, 14, 16]   # padded key-tile extent per slot

_cache = {}


def _build():
    nc = bacc.Bacc("TRN2", target_bir_lowering=False, debug=False,
                   enable_asserts=False, num_devices=8)
    def din(name, shape, dt=BF16):
        return nc.dram_tensor(name, list(shape), dt, kind="ExternalInput").ap()

    xt_d = din("xt", (C, T))            # x[b].T
    xq_d = din("xq", (C, 1024))         # own q columns of xt, slot-sorted
    wq_d = din("wq", (C, C))            # pre-scaled by 1/8
    wk_d = din("wk", (C, C))
    wv_d = din("wv", (C, C))
    wp_d = din("wp", (C, C))
    bq_d = din("bq", (8, 128, 1), F32)  # pre-scaled by 1/8
    bk_d = din("bk", (8, 128, 1), F32)
    bpeb_d = din("bpeb", (128, C), F32)
    mk_d = din("masks", (128, 16 * 128))
    y_d = nc.dram_tensor("y", [1024, C], F32, kind="ExternalOutput").ap()

    bypass = mybir.AluOpType.bypass
    mult = mybir.AluOpType.mult
    add = mybir.AluOpType.add
    EXP = mybir.ActivationFunctionType.Exp

    with tile.TileContext(nc) as tc:
        # ---------------- persistent tiles ------------------------------
        per = tc.alloc_tile_pool(name="per", bufs=1)
        KT = [per.tile([128, T], BF16, name=f"kt{j}", tag=f"kt{j}") for j in range(8)]
        VA = [per.tile([128, VA_W], BF16, name=f"va{m}", tag=f"va{m}") for m in range(16)]
        QT = [per.tile([128, 1024], BF16, name=f"qt{j}", tag=f"qt{j}") for j in range(8)]
        OT = [per.tile([128, 1024], BF16, name=f"ot{j}", tag=f"ot{j}") for j in range(8)]
        MK = per.tile([128, 16 * 128], BF16, name="mk", tag="mk")
        bks = [per.tile([128, 1], F32, name=f"bks{j}", tag=f"bks{j}") for j in range(8)]
        bqs = [per.tile([128, 1], F32, name=f"bqs{j}", tag=f"bqs{j}") for j in range(8)]
        bpeb = per.tile([128, C], F32, name="bpeb", tag="bpeb")
        ones16 = per.tile([128, H], BF16, name="ones16", tag="ones16")

        wkvp = tc.alloc_tile_pool(name="wkvp", bufs=1)
        wkc = [wkvp.tile([128, C], BF16, name=f"wkc{c}", tag=f"wkc{c}") for c in range(8)]
        wvc = [wkvp.tile([128, C], BF16, name=f"wvc{c}", tag=f"wvc{c}") for c in range(8)]
        wpp = tc.alloc_tile_pool(name="wpp", bufs=1)
        wpc = [wpp.tile([128, C], BF16, name=f"wpc{c}", tag=f"wpc{c}") for c in range(8)]
        xs1p = tc.alloc_tile_pool(name="xs1p", bufs=1)
        xts1 = [xs1p.tile([128, 1024], BF16, name=f"x1_{c}", tag=f"x1_{c}")
                for c in range(8)]

        def k_block(ps1, xts, j, ss):
            """K^T block j for one t-superslab (16 matmuls + Act evac)."""
            pk = ps1.tile([128, 1024], F32, name="pk", tag="pk")
            for c in range(8):
                for sub in range(2):
                    nc.tensor.matmul(out=pk[:, 512*sub:512*(sub+1)],
                                     lhsT=wkc[c][:, 128*j:128*(j+1)],
                                     rhs=xts[c][:, 512*sub:512*(sub+1)],
                                     start=(c == 0), stop=(c == 7))
            nc.scalar.add(out=KT[j][:, 1024*ss:1024*(ss+1)], in_=pk[:],
                          add=bks[j][:])

        def v_block(ps1, xts, tt, ss):
            """V_aug t-tile m=8*ss+tt (16 matmuls + Act evac, rearranged)."""
            pv = ps1.tile([128, 1024], F32, name="pv", tag="pk")
            for c in range(8):
                for jc in range(2):
                    nc.tensor.matmul(out=pv[:, 512*jc:512*(jc+1)],
                                     lhsT=xts[c][:, 128*tt:128*(tt+1)],
                                     rhs=wvc[c][:, 512*jc:512*(jc+1)],
                                     start=(c == 0), stop=(c == 7))
            dst = VA[8*ss+tt][:].rearrange("p (h d) -> p h d", d=D+1)[:, :, 0:D]
            src = pv[:].rearrange("p (h d) -> p h d", d=D)
            nc.scalar.copy(out=dst, in_=src)

        # -------- part 1: superslab-0 K^T and V, all of Q^T -------------
        with tc.tile_pool(name="wqp", bufs=1) as wqp, \
             tc.tile_pool(name="xs0p", bufs=1) as xs0p, \
             tc.tile_pool(name="ps1", bufs=3, space="PSUM") as ps1:
            wqc = [wqp.tile([128, C], BF16, name=f"wqc{c}", tag=f"wqc{c}")
                   for c in range(8)]
            # priority DMAs: interleave K weights with x superslab 0
            xts0 = []
            for c in range(8):
                nc.sync.dma_start(out=wkc[c][:], in_=wk_d[128*c:128*(c+1), :])
                xt_t = xs0p.tile([128, 1024], BF16, name=f"x0_{c}", tag=f"x0_{c}")
                nc.sync.dma_start(out=xt_t[:], in_=xt_d[128*c:128*(c+1), 0:1024])
                xts0.append(xt_t)
            for j in range(8):
                nc.sync.dma_start(out=bks[j][:], in_=bk_d[j])
            for c in range(8):
                nc.sync.dma_start(out=wvc[c][:], in_=wv_d[128*c:128*(c+1), :])
            for c in range(8):
                nc.sync.dma_start(out=wqc[c][:], in_=wq_d[128*c:128*(c+1), :])
            for j in range(8):
                nc.sync.dma_start(out=bqs[j][:], in_=bq_d[j])
            nc.sync.dma_start(out=MK[:], in_=mk_d[:])
            nc.sync.dma_start(out=bpeb[:], in_=bpeb_d[:])
            for c in range(8):
                nc.sync.dma_start(out=xts1[c][:], in_=xt_d[128*c:128*(c+1), 1024:2048])
            # xq reuses the xts0 buffers (second tile per tag)
            xqc = []
            for c in range(8):
                xq_t = xs0p.tile([128, 1024], BF16, name=f"x0_{c}", tag=f"x0_{c}")
                nc.sync.dma_start(out=xq_t[:], in_=xq_d[128*c:128*(c+1), :])
                xqc.append(xq_t)

            nc.vector.memset(ones16[:], 1.0)
            ones16_3d = ones16[:].unsqueeze(2)
            for m in range(16):
                dst1 = VA[m][:].rearrange("p (h d) -> p h d", d=D+1)[:, :, D:D+1]
                nc.vector.tensor_copy(out=dst1, in_=ones16_3d)

            for j in range(8):
                k_block(ps1, xts0, j, 0)
            for tt in range(8):
                v_block(ps1, xts0, tt, 0)
            for j in range(8):
                pq = ps1.tile([128, 1024], F32, name="pq", tag="pk")
                for c in range(8):
                    for sub in range(2):
                        nc.tensor.matmul(out=pq[:, 512*sub:512*(sub+1)],
                                         lhsT=wqc[c][:, 128*j:128*(j+1)],
                                         rhs=xqc[c][:, 512*sub:512*(sub+1)],
                                         start=(c == 0), stop=(c == 7))
                nc.scalar.add(out=QT[j][:, :], in_=pq[:], add=bqs[j][:])

        # ---------------- attention helpers -----------------------------
        def emit_pair_packed(ps, att, ss_bufs, j, p, g):
            m0, m1 = 2*g, 2*g + 1
            so = max(0, g - 4*p)
            qc0 = 512*p + 128*so
            N = 512 - 128*so
            masked = (g >= 4*p)
            halves = [(0, [(0, m0), (1, m1)]), (1, [(2, m0), (3, m1)])]
            ss_t = ps.tile([128, 1024], F32, name="ss", tag="ss", bufs=ss_bufs)
            # adjacent quadrant pairs (h0/h64) stream concurrently; their
            # dsts land in different PSUM banks (u0/u2, u1/u3)
            for ui in range(2):
                for half in range(2):
                    r0, r1 = 64*half, 64*(half+1)
                    u = 2*half + ui
                    m = (m0, m1)[ui]
                    nc.tensor.matmul(out=ss_t[:, N*u:N*(u+1)],
                                     lhsT=KT[j][r0:r1, 128*m:128*(m+1)],
                                     rhs=QT[j][r0:r1, qc0:qc0+N],
                                     tile_position=(r0, 0),
                                     start=True, stop=True)
            pt = att.tile([128, 1024], BF16, name="pt", tag="pt", bufs=10)
            nc.scalar.activation(out=pt[:, 0:4*N], in_=ss_t[:, 0:4*N], func=EXP)
            out = []
            for half, ums in halves:
                if masked:
                    for u, m in ums:
                        nc.vector.scalar_tensor_tensor(
                            out=pt[:, N*u:N*u+128], in0=pt[:, N*u:N*u+128],
                            scalar=0.0, in1=MK[:, 128*m:128*(m+1)],
                            op0=bypass, op1=mult)
                out.append((pt, N, so, ums, half))
            return out

        def emit_pair_unpacked(ps, att, ss_bufs, j, p, g):
            m0, m1 = 2*g, 2*g + 1
            so = max(0, g - 4*p)
            qc0 = 512*p + 128*so
            N = 512 - 128*so
            masked = (g >= 4*p)
            sst = [ps.tile([128, 1024], F32, name="ss", tag="ss", bufs=ss_bufs)
                   for _ in range(2)]
            # u-major, half-inner: adjacent quadrant pairs (h0/h64) stream
            # concurrently into different ss tiles (different banks)
            for u, m in ((0, m0), (1, m1)):
                for half in range(2):
                    r0, r1 = 64*half, 64*(half+1)
                    c0 = N * u           # split dst at PSUM bank boundaries
                    while c0 < N * (u + 1):
                        c1 = min(N * (u + 1), (c0 // 512 + 1) * 512)
                        nc.tensor.matmul(out=sst[half][:, c0:c1],
                                         lhsT=KT[j][r0:r1, 128*m:128*(m+1)],
                                         rhs=QT[j][r0:r1, qc0 + c0 - N*u:
                                                          qc0 + c1 - N*u],
                                         tile_position=(r0, 0),
                                         start=True, stop=True)
                        c0 = c1
            out = []
            for half in range(2):
                pt = att.tile([128, 1024], BF16, name="pt", tag="pt", bufs=10)
                nc.scalar.activation(out=pt[:, 0:2*N], in_=sst[half][:, 0:2*N],
                                     func=EXP)
                if masked:
                    for u, m in ((0, m0), (1, m1)):
                        nc.vector.scalar_tensor_tensor(
                            out=pt[:, N*u:N*u+128], in0=pt[:, N*u:N*u+128],
                            scalar=0.0, in1=MK[:, 128*m:128*(m+1)],
                            op0=bypass, op1=mult)
                out.append((pt, N, so, [(0, m0), (1, m1)], half))
            return out

        def emit_attn_pair(ps, att, ss_bufs, j, p, g):
            so = max(0, g - 4*p)
            N = 512 - 128*so
            if 4*N == 1024:     # packed halves land in separate PSUM banks
                return emit_pair_packed(ps, att, ss_bufs, j, p, g)
            return emit_pair_unpacked(ps, att, ss_bufs, j, p, g)

        def emit_pv(acc, j, mlast, items):
            for (ppt, pN, pso, ums, phalf) in items:
                hh = 2*j + phalf
                for u, m in ums:
                    nc.tensor.matmul(out=acc[phalf][:, 128*pso:512],
                                     lhsT=VA[m][:, 65*hh:65*(hh+1)],
                                     rhs=ppt[:, pN*u:pN*(u+1)],
                                     start=(m == 0), stop=(m == mlast),
                                     skip_group_check=True)

        def emit_norm(att, acc, j, p):
            for half in range(2):
                lsb = att.tile([1, 512], F32, name="lsb", tag=f"lsb{half}", bufs=1)
                nc.vector.tensor_copy(out=lsb[:], in_=acc[half][64:65, :])
                rsb = att.tile([1, 512], F32, name="rsb", tag=f"rsb{half}", bufs=1)
                nc.vector.reciprocal_approx_fast(rsb[:], lsb[:])
                rbb = att.tile([64, 512], F32, name="rbb", tag=f"rbb{half}", bufs=1)
                nc.gpsimd.partition_broadcast(rbb[:], rsb[:])
                nc.vector.scalar_tensor_tensor(
                    out=OT[j][64*half:64*(half+1), 512*p:512*(p+1)],
                    in0=acc[half][0:64, :], scalar=0.0, in1=rbb[:],
                    op0=bypass, op1=mult)

        LAG = 2
        # ---- X sweep: pass 0 for all j; superslab-1 K/V as PE filler ----
        with tc.tile_pool(name="attx", bufs=1) as attx, \
             tc.tile_pool(name="psx", bufs=1, space="PSUM") as psx:
            for c in range(8):
                nc.sync.dma_start(out=wpc[c][:], in_=wp_d[128*c:128*(c+1), :])
            filler = []
            for j in range(8):
                filler.append(("k", j))
            for tt in range(8):
                filler.append(("v", tt))
            fi = 0
            for j in range(8):
                acc = [psx.tile([65, 512], F32, name=f"acc{h}", tag=f"acc{h}")
                       for h in range(2)]
                pend = []
                for g in range(4):
                    pend.append(emit_attn_pair(psx, attx, 2, j, 0, g))
                    if fi < len(filler):   # one filler block per pair
                        kind, idx = filler[fi]; fi += 1
                        if kind == "k":
                            k_block(psx, xts1, idx, 1)
                        else:
                            v_block(psx, xts1, idx, 1)
                    if len(pend) > LAG:
                        emit_pv(acc, j, 7, pend.pop(0))
                for items in pend:
                    emit_pv(acc, j, 7, items)
                emit_norm(attx, acc, j, 0)
            while fi < len(filler):
                kind, idx = filler[fi]; fi += 1
                if kind == "k":
                    k_block(psx, xts1, idx, 1)
                else:
                    v_block(psx, xts1, idx, 1)
        xs1p.release()

        # ---- Y sweep: pass 1, two j's interleaved ----------------------
        with tc.tile_pool(name="atty", bufs=1) as atty:
            with tc.tile_pool(name="psy", bufs=1, space="PSUM") as psy:
                for jp in range(4):
                    js = (2*jp, 2*jp + 1)
                    acc = {jj: [psy.tile([65, 512], F32, name=f"acc{jj%2}{h}",
                                         tag=f"acc{jj%2}{h}") for h in range(2)]
                           for jj in js}
                    pend = []
                    for g in range(8):
                        for jj in js:
                            pend.append((jj, emit_attn_pair(psy, atty, 2, jj, 1, g)))
                            if len(pend) > 2*LAG - 1:
                                pj, items = pend.pop(0)
                                emit_pv(acc[pj], pj, 15, items)
                    for pj, items in pend:
                        emit_pv(acc[pj], pj, 15, items)
                    for jj in js:
                        emit_norm(atty, acc[jj], jj, 1)
            # ---- projection tail (all 8 q-tiles) ----
            with tc.tile_pool(name="psp", bufs=1, space="PSUM") as psp:
                for ti in range(8):
                    py = [psp.tile([128, 512], F32, name=f"py{jc}",
                                   tag=f"py{jc}", bufs=2) for jc in range(2)]
                    for ci in range(8):
                        for jc in range(2):
                            nc.tensor.matmul(out=py[jc][:],
                                             lhsT=OT[ci][:, 128*ti:128*(ti+1)],
                                             rhs=wpc[ci][:, 512*jc:512*(jc+1)],
                                             start=(ci == 0), stop=(ci == 7))
                    for jc in range(2):
                        ysb = atty.tile([128, 512], F32, name="ysb", tag="ysb",
                                        bufs=4)
                        nc.vector.scalar_tensor_tensor(
                            out=ysb[:], in0=py[jc][:], scalar=0.0,
                            in1=bpeb[:, 512*jc:512*(jc+1)],
                            op0=bypass, op1=add)
                        nc.sync.dma_start(
                            out=y_d[128*ti:128*(ti+1), 512*jc:512*(jc+1)],
                            in_=ysb[:])
        wpp.release()
        wkvp.release()
        per.release()

    nc.compile()
    return nc


def _get_nc():
    if "nc" not in _cache:
        _cache["nc"] = _build()
    return _cache["nc"]


def _host_prep(x, Wqkv, bqkv, Wproj, bproj):
    x = np.ascontiguousarray(np.asarray(x, dtype=np.float32))
    Wqkv = np.asarray(Wqkv, dtype=np.float32)
    bqkv = np.asarray(bqkv, dtype=np.float32)
    Wproj = np.ascontiguousarray(np.asarray(Wproj, dtype=np.float32))
    bproj = np.asarray(bproj, dtype=np.float32)

    wq = np.ascontiguousarray(Wqkv[:, :C] * np.float32(0.125)).astype(NPBF)
    wk = np.ascontiguousarray(Wqkv[:, C:2*C]).astype(NPBF)
    wv = np.ascontiguousarray(Wqkv[:, 2*C:]).astype(NPBF)
    wp = Wproj.astype(NPBF)
    bq8 = (bqkv[:C] * np.float32(0.125)).reshape(8, 128, 1).copy()
    bk8 = bqkv[C:2*C].reshape(8, 128, 1).copy()
    bv = bqkv[2*C:]
    bpe = (bproj.astype(np.float64) + bv.astype(np.float64) @ Wproj.astype(np.float64)).astype(np.float32)
    bpeb = np.ascontiguousarray(np.broadcast_to(bpe, (128, C)))

    ridx = np.arange(128)[:, None]
    cidx = np.arange(128)[None, :]
    tri = (ridx <= cidx)
    masks = []
    for par in range(2):
        mk = np.zeros((128, 16 * 128), dtype=NPBF)
        for m in range(16):
            g = OWNT[par][m // 2]
            if m < g:
                mk[:, 128*m:128*(m+1)] = 1
            elif m == g:
                mk[:, 128*m:128*(m+1)] = tri
        masks.append(mk)

    in_maps = []
    for core in range(8):
        b, par = core // 2, core % 2
        xt = np.ascontiguousarray(x[b].T.astype(NPBF))
        xq = np.ascontiguousarray(
            np.concatenate([xt[:, 128*g:128*(g+1)] for g in OWNT[par]], axis=1))
        in_maps.append(dict(xt=xt, xq=xq, wq=wq, wk=wk, wv=wv, wp=wp,
                            bq=bq8, bk=bk8, bpeb=bpeb, masks=masks[par]))
    return in_maps


def kernel(x, Wqkv, bqkv, Wproj, bproj):
    nc = _get_nc()
    in_maps = _host_prep(x, Wqkv, bqkv, Wproj, bproj)
    trace = bool(os.environ.get("BASS_TRACE"))
    res = run_bass_kernel_spmd(nc, in_maps, list(range(8)), trace=trace)
    _cache["last_exec_time_ns"] = res.exec_time_ns
    _cache["last_res"] = res
    out = np.empty((B, T, C), dtype=np.float32)
    for core in range(8):
        b, par = core // 2, core % 2
        y = res.results[core]["y"]
        for si, g in enumerate(OWNT[par]):
            out[b, 128*g:128*(g+1)] = y[128*si:128*(si+1)]
    return out


# revision 20
# speedup vs baseline: 1.6387x; 1.0967x over previous
"""Causal self-attention on 8 TRN2 NeuronCores (Bass/Tile, SPMD).

Problem: B=4, T=2048, C=1024, H=16, D=64, fp32 in/out.

Sharding: core i = (batch b=i//2, parity p=i%2). Each core computes ALL 16
heads for 8 of the 16 128-wide query tiles of its batch: parity 0 owns
global q-tiles {0,3,4,7,8,11,12,15}, parity 1 owns {1,2,5,6,9,10,13,14}.
Sorted by causal extent, both parities pad to the SAME per-slot key-tile
extents E = [2,4,6,8,10,12,14,16] -> every core runs the IDENTICAL
instruction stream (SPMD); causality handled by host-supplied mask data
(mask m is triangular/ones/zeros depending on parity). K/V are computed
for the full sequence on both cores of a batch.

All tensor data is bf16 (PSUM accumulation fp32). Schedule is built to
keep the PE continuously busy (it only reaches its 2.4 GHz p-state after
~3us of uninterrupted work):
  part 1: K^T/V for t-superslab 0 + all of Q^T (j-major matmuls, weights
          stationary; PSUM evacuated with +bias on the Act engine).
  X sweep: attention pass 0 (slots 0-3, key tiles < 8) for all j,
          software-pipelined (PV lags S/exp by 2 pairs), with superslab-1
          K^T/V matmuls interleaved as filler so exp latency never idles
          the PE.
  Y sweep: attention pass 1 (slots 4-7, all 16 key tiles), two j's
          interleaved (independent chains) as mutual filler.
  tail:   output projection Y = O^T.T @ Wproj + bias_eff from SBUF
          (bias_eff = bproj + bv @ Wproj, exact since softmax rows sum
          to 1).

Attention inner scheme: q-tiles needing key-tile m form a contiguous
suffix of the slot-sorted Q columns, so S^T(m) = K_m^T Q_suffix is ONE
long matmul (N=512..128) per (half, m); matmul PSUM outputs are split at
512-col PSUM bank boundaries (HW corrupts writes that cross a bank);
batched exp on ScalarE; per-pair diagonal masks on DVE; PV accumulates
into per-pass [65,512] PSUM accumulators (row 64 = softmax denominator);
normalize via DVE reciprocal + GpSimd partition broadcast; O^T in SBUF.

Host: slices/transposes/casts inputs to bf16, precomputes masks +
effective bias, reassembles the 8 per-core [1024,1024] outputs.
"""
import os
import numpy as np
import ml_dtypes

import concourse.bacc as bacc
import concourse.mybir as mybir
import concourse.tile as tile
from concourse.bass_utils import run_bass_kernel_spmd

B, T, C, H, D = 4, 2048, 1024, 16, 64
F32 = mybir.dt.float32
BF16 = mybir.dt.bfloat16
NPBF = ml_dtypes.bfloat16
VA_W = H * (D + 1)            # 1040: V_aug cols = 16 heads x (64 + ones)
OWNT = [[0, 3, 4, 7, 8, 11, 12, 15], [1, 2, 5, 6, 9, 10, 13, 14]]
EXT = [2, 4, 6, 8, 10, 12, 14, 16]   # padded key-tile extent per slot

_cache = {}


def _build():
    nc = bacc.Bacc("TRN2", target_bir_lowering=False, debug=False,
                   enable_asserts=False, num_devices=8)
    def din(name, shape, dt=BF16):
        return nc.dram_tensor(name, list(shape), dt, kind="ExternalInput").ap()

    xt_d = din("xt", (C, T))            # x[b].T
    xq_d = din("xq", (C, 1024))         # own q columns of xt, slot-sorted
    wq_d = din("wq", (C, C))            # pre-scaled by 1/8
    wk_d = din("wk", (C, C))
    wv_d = din("wv", (C, C))
    wp_d = din("wp", (C, C))
    bq_d = din("bq", (8, 128, 1), F32)  # pre-scaled by 1/8
    bk_d = din("bk", (8, 128, 1), F32)
    bpeb_d = din("bpeb", (128, C), F32)
    mk_d = din("masks", (128, 16 * 128))
    y_d = nc.dram_tensor("y", [1024, C], F32, kind="ExternalOutput").ap()

    bypass = mybir.AluOpType.bypass
    mult = mybir.AluOpType.mult
    add = mybir.AluOpType.add
    EXP = mybir.ActivationFunctionType.Exp

    with tile.TileContext(nc) as tc:
        # ---------------- persistent tiles ------------------------------
        per = tc.alloc_tile_pool(name="per", bufs=1)
        KT = [per.tile([128, T], BF16, name=f"kt{j}", tag=f"kt{j}") for j in range(8)]
        VA = [per.tile([128, VA_W], BF16, name=f"va{m}", tag=f"va{m}") for m in range(16)]
        QT = [per.tile([128, 1024], BF16, name=f"qt{j}", tag=f"qt{j}") for j in range(8)]
        OT = [per.tile([128, 1024], BF16, name=f"ot{j}", tag=f"ot{j}") for j in range(8)]
        MK = per.tile([128, 16 * 128], BF16, name="mk", tag="mk")
        bks = [per.tile([128, 1], F32, name=f"bks{j}", tag=f"bks{j}") for j in range(8)]
        bqs = [per.tile([128, 1], F32, name=f"bqs{j}", tag=f"bqs{j}") for j in range(8)]
        bpeb = per.tile([128, C], F32, name="bpeb", tag="bpeb")
        ones16 = per.tile([128, H], BF16, name="ones16", tag="ones16")

        wkvp = tc.alloc_tile_pool(name="wkvp", bufs=1)
        wkc = [wkvp.tile([128, C], BF16, name=f"wkc{c}", tag=f"wkc{c}") for c in range(8)]
        wvc = [wkvp.tile([128, C], BF16, name=f"wvc{c}", tag=f"wvc{c}") for c in range(8)]
        wpp = tc.alloc_tile_pool(name="wpp", bufs=1)
        wpc = [wpp.tile([128, C], BF16, name=f"wpc{c}", tag=f"wpc{c}") for c in range(8)]
        xs1p = tc.alloc_tile_pool(name="xs1p", bufs=1)
        xts1 = [xs1p.tile([128, 1024], BF16, name=f"x1_{c}", tag=f"x1_{c}")
                for c in range(8)]

        def k_block(ps1, xts, j, ss):
            """K^T block j for one t-superslab (16 matmuls + Act evac)."""
            pk = ps1.tile([128, 1024], F32, name="pk", tag="pk")
            for c in range(8):
                for sub in range(2):
                    nc.tensor.matmul(out=pk[:, 512*sub:512*(sub+1)],
                                     lhsT=wkc[c][:, 128*j:128*(j+1)],
                                     rhs=xts[c][:, 512*sub:512*(sub+1)],
                                     start=(c == 0), stop=(c == 7))
            nc.scalar.add(out=KT[j][:, 1024*ss:1024*(ss+1)], in_=pk[:],
                          add=bks[j][:])

        def v_block(ps1, xts, tt, ss):
            """V_aug t-tile m=8*ss+tt (16 matmuls + Act evac, rearranged)."""
            pv = ps1.tile([128, 1024], F32, name="pv", tag="pk")
            for c in range(8):
                for jc in range(2):
                    nc.tensor.matmul(out=pv[:, 512*jc:512*(jc+1)],
                                     lhsT=xts[c][:, 128*tt:128*(tt+1)],
                                     rhs=wvc[c][:, 512*jc:512*(jc+1)],
                                     start=(c == 0), stop=(c == 7))
            dst = VA[8*ss+tt][:].rearrange("p (h d) -> p h d", d=D+1)[:, :, 0:D]
            src = pv[:].rearrange("p (h d) -> p h d", d=D)
            nc.scalar.copy(out=dst, in_=src)

        # -------- part 1: superslab-0 K^T and V, all of Q^T -------------
        with tc.tile_pool(name="wqp", bufs=1) as wqp, \
             tc.tile_pool(name="xs0p", bufs=1) as xs0p, \
             tc.tile_pool(name="ps1", bufs=3, space="PSUM") as ps1:
            wqc = [wqp.tile([128, C], BF16, name=f"wqc{c}", tag=f"wqc{c}")
                   for c in range(8)]
            # priority DMAs: interleave K weights with x superslab 0
            xts0 = []
            for c in range(8):
                nc.sync.dma_start(out=wkc[c][:], in_=wk_d[128*c:128*(c+1), :])
                xt_t = xs0p.tile([128, 1024], BF16, name=f"x0_{c}", tag=f"x0_{c}")
                nc.sync.dma_start(out=xt_t[:], in_=xt_d[128*c:128*(c+1), 0:1024])
                xts0.append(xt_t)
            for j in range(8):
                nc.sync.dma_start(out=bks[j][:], in_=bk_d[j])
            for c in range(8):
                nc.sync.dma_start(out=wvc[c][:], in_=wv_d[128*c:128*(c+1), :])
            for c in range(8):
                nc.sync.dma_start(out=wqc[c][:], in_=wq_d[128*c:128*(c+1), :])
            for j in range(8):
                nc.sync.dma_start(out=bqs[j][:], in_=bq_d[j])
            nc.sync.dma_start(out=MK[:], in_=mk_d[:])
            nc.sync.dma_start(out=bpeb[:], in_=bpeb_d[:])
            for c in range(8):
                nc.sync.dma_start(out=xts1[c][:], in_=xt_d[128*c:128*(c+1), 1024:2048])
            # xq reuses the xts0 buffers (second tile per tag)
            xqc = []
            for c in range(8):
                xq_t = xs0p.tile([128, 1024], BF16, name=f"x0_{c}", tag=f"x0_{c}")
                nc.sync.dma_start(out=xq_t[:], in_=xq_d[128*c:128*(c+1), :])
                xqc.append(xq_t)

            nc.vector.memset(ones16[:], 1.0)
            ones16_3d = ones16[:].unsqueeze(2)
            for m in range(16):
                dst1 = VA[m][:].rearrange("p (h d) -> p h d", d=D+1)[:, :, D:D+1]
                nc.vector.tensor_copy(out=dst1, in_=ones16_3d)

            for j in range(8):
                k_block(ps1, xts0, j, 0)
            for tt in range(8):
                v_block(ps1, xts0, tt, 0)
            for j in range(8):
                pq = ps1.tile([128, 1024], F32, name="pq", tag="pk")
                for c in range(8):
                    for sub in range(2):
                        nc.tensor.matmul(out=pq[:, 512*sub:512*(sub+1)],
                                         lhsT=wqc[c][:, 128*j:128*(j+1)],
                                         rhs=xqc[c][:, 512*sub:512*(sub+1)],
                                         start=(c == 0), stop=(c == 7))
                nc.scalar.add(out=QT[j][:, :], in_=pq[:], add=bqs[j][:])

        # ---------------- attention helpers -----------------------------
        def emit_pair_packed(ps, att, ss_bufs, j, p, g):
            m0, m1 = 2*g, 2*g + 1
            so = max(0, g - 4*p)
            qc0 = 512*p + 128*so
            N = 512 - 128*so
            masked = (g >= 4*p)
            halves = [(0, [(0, m0), (1, m1)]), (1, [(2, m0), (3, m1)])]
            ss_t = ps.tile([128, 1024], F32, name="ss", tag="ss", bufs=ss_bufs)
            # adjacent quadrant pairs (h0/h64) stream concurrently; their
            # dsts land in different PSUM banks (u0/u2, u1/u3)
            for ui in range(2):
                for half in range(2):
                    r0, r1 = 64*half, 64*(half+1)
                    u = 2*half + ui
                    m = (m0, m1)[ui]
                    nc.tensor.matmul(out=ss_t[:, N*u:N*(u+1)],
                                     lhsT=KT[j][r0:r1, 128*m:128*(m+1)],
                                     rhs=QT[j][r0:r1, qc0:qc0+N],
                                     tile_position=(r0, 0),
                                     start=True, stop=True)
            pt = att.tile([128, 1024], BF16, name="pt", tag="pt", bufs=10)
            nc.scalar.activation(out=pt[:, 0:4*N], in_=ss_t[:, 0:4*N], func=EXP)
            out = []
            for half, ums in halves:
                if masked:
                    for u, m in ums:
                        nc.vector.scalar_tensor_tensor(
                            out=pt[:, N*u:N*u+128], in0=pt[:, N*u:N*u+128],
                            scalar=0.0, in1=MK[:, 128*m:128*(m+1)],
                            op0=bypass, op1=mult)
                out.append((pt, N, so, ums, half))
            return out

        def emit_pair_unpacked(ps, att, ss_bufs, j, p, g):
            m0, m1 = 2*g, 2*g + 1
            so = max(0, g - 4*p)
            qc0 = 512*p + 128*so
            N = 512 - 128*so
            masked = (g >= 4*p)
            sst = [ps.tile([128, 1024], F32, name="ss", tag="ss", bufs=ss_bufs)
                   for _ in range(2)]
            # u-major, half-inner: adjacent quadrant pairs (h0/h64) stream
            # concurrently into different ss tiles (different banks)
            for u, m in ((0, m0), (1, m1)):
                for half in range(2):
                    r0, r1 = 64*half, 64*(half+1)
                    c0 = N * u           # split dst at PSUM bank boundaries
                    while c0 < N * (u + 1):
                        c1 = min(N * (u + 1), (c0 // 512 + 1) * 512)
                        nc.tensor.matmul(out=sst[half][:, c0:c1],
                                         lhsT=KT[j][r0:r1, 128*m:128*(m+1)],
                                         rhs=QT[j][r0:r1, qc0 + c0 - N*u:
                                                          qc0 + c1 - N*u],
                                         tile_position=(r0, 0),
                                         start=True, stop=True)
                        c0 = c1
            out = []
            for half in range(2):
                pt = att.tile([128, 1024], BF16, name="pt", tag="pt", bufs=10)
                nc.scalar.activation(out=pt[:, 0:2*N], in_=sst[half][:, 0:2*N],
                                     func=EXP)
                if masked:
                    for u, m in ((0, m0), (1, m1)):
                        nc.vector.scalar_tensor_tensor(
                            out=pt[:, N*u:N*u+128], in0=pt[:, N*u:N*u+128],
                            scalar=0.0, in1=MK[:, 128*m:128*(m+1)],
                            op0=bypass, op1=mult)
                out.append((pt, N, so, [(0, m0), (1, m1)], half))
            return out

        def emit_attn_pair(ps, att, ss_bufs, j, p, g):
            so = max(0, g - 4*p)
            N = 512 - 128*so
            if 4*N == 1024:     # packed halves land in separate PSUM banks
                return emit_pair_packed(ps, att, ss_bufs, j, p, g)
            return emit_pair_unpacked(ps, att, ss_bufs, j, p, g)

        def emit_pv(acc, j, mlast, items):
            for (ppt, pN, pso, ums, phalf) in items:
                hh = 2*j + phalf
                for u, m in ums:
                    nc.tensor.matmul(out=acc[phalf][:, 128*pso:512],
                                     lhsT=VA[m][:, 65*hh:65*(hh+1)],
                                     rhs=ppt[:, pN*u:pN*(u+1)],
                                     start=(m == 0), stop=(m == mlast),
                                     skip_group_check=True)

        def emit_norm(att, acc, j, p):
            for half in range(2):
                lsb = att.tile([1, 512], F32, name="lsb", tag=f"lsb{half}", bufs=1)
                nc.vector.tensor_copy(out=lsb[:], in_=acc[half][64:65, :])
                rsb = att.tile([1, 512], F32, name="rsb", tag=f"rsb{half}", bufs=1)
                nc.vector.reciprocal_approx_fast(rsb[:], lsb[:])
                rbb = att.tile([64, 512], F32, name="rbb", tag=f"rbb{half}", bufs=1)
                nc.gpsimd.partition_broadcast(rbb[:], rsb[:])
                nc.vector.scalar_tensor_tensor(
                    out=OT[j][64*half:64*(half+1), 512*p:512*(p+1)],
                    in0=acc[half][0:64, :], scalar=0.0, in1=rbb[:],
                    op0=bypass, op1=mult)

        LAG = 2
        # ---- X sweep: pass 0 for all j; superslab-1 K/V as PE filler ----
        with tc.tile_pool(name="attx", bufs=1) as attx, \
             tc.tile_pool(name="psx", bufs=1, space="PSUM") as psx:
            for c in range(8):
                nc.sync.dma_start(out=wpc[c][:], in_=wp_d[128*c:128*(c+1), :])
            def filler_steps():
                # superslab-1 K/V in half-block steps (8 matmuls per step)
                for j in range(8):
                    pk = psx.tile([128, 1024], F32, name="pk", tag="pk")
                    for c in range(8):
                        for sub in range(2):
                            nc.tensor.matmul(out=pk[:, 512*sub:512*(sub+1)],
                                             lhsT=wkc[c][:, 128*j:128*(j+1)],
                                             rhs=xts1[c][:, 512*sub:512*(sub+1)],
                                             start=(c == 0), stop=(c == 7))
                        if c == 3:
                            yield
                    nc.scalar.add(out=KT[j][:, 1024:2048], in_=pk[:], add=bks[j][:])
                    yield
                for tt in range(8):
                    pv = psx.tile([128, 1024], F32, name="pv", tag="pk")
                    for c in range(8):
                        for jc in range(2):
                            nc.tensor.matmul(out=pv[:, 512*jc:512*(jc+1)],
                                             lhsT=xts1[c][:, 128*tt:128*(tt+1)],
                                             rhs=wvc[c][:, 512*jc:512*(jc+1)],
                                             start=(c == 0), stop=(c == 7))
                        if c == 3:
                            yield
                    dst = VA[8+tt][:].rearrange("p (h d) -> p h d", d=D+1)[:, :, 0:D]
                    src = pv[:].rearrange("p (h d) -> p h d", d=D)
                    nc.scalar.copy(out=dst, in_=src)
                    yield
            fgen = filler_steps()
            for j in range(8):
                acc = [psx.tile([65, 512], F32, name=f"acc{h}", tag=f"acc{h}")
                       for h in range(2)]
                pend = []
                for g in range(4):
                    pend.append(emit_attn_pair(psx, attx, 2, j, 0, g))
                    next(fgen, None)
                    if len(pend) > LAG:
                        emit_pv(acc, j, 7, pend.pop(0))
                for items in pend:
                    emit_pv(acc, j, 7, items)
                emit_norm(attx, acc, j, 0)
            for _ in fgen:
                pass
        xs1p.release()

        # ---- Y sweep: pass 1, two j's interleaved ----------------------
        with tc.tile_pool(name="atty", bufs=1) as atty:
            with tc.tile_pool(name="psy", bufs=1, space="PSUM") as psy:
                for jp in range(4):
                    js = (2*jp, 2*jp + 1)
                    acc = {jj: [psy.tile([65, 512], F32, name=f"acc{jj%2}{h}",
                                         tag=f"acc{jj%2}{h}") for h in range(2)]
                           for jj in js}
                    pend = []
                    for g in range(8):
                        for jj in js:
                            pend.append((jj, emit_attn_pair(psy, atty, 2, jj, 1, g)))
                            if len(pend) > 2*LAG - 1:
                                pj, items = pend.pop(0)
                                emit_pv(acc[pj], pj, 15, items)
                    for pj, items in pend:
                        emit_pv(acc[pj], pj, 15, items)
                    for jj in js:
                        emit_norm(atty, acc[jj], jj, 1)
            # ---- projection tail (all 8 q-tiles) ----
            with tc.tile_pool(name="psp", bufs=1, space="PSUM") as psp:
                for ti in range(8):
                    py = [psp.tile([128, 512], F32, name=f"py{jc}",
                                   tag=f"py{jc}", bufs=2) for jc in range(2)]
                    for ci in range(8):
                        for jc in range(2):
                            nc.tensor.matmul(out=py[jc][:],
                                             lhsT=OT[ci][:, 128*ti:128*(ti+1)],
                                             rhs=wpc[ci][:, 512*jc:512*(jc+1)],
                                             start=(ci == 0), stop=(ci == 7))
                    for jc in range(2):
                        ysb = atty.tile([128, 512], F32, name="ysb", tag="ysb",
                                        bufs=4)
                        nc.vector.scalar_tensor_tensor(
                            out=ysb[:], in0=py[jc][:], scalar=0.0,
                            in1=bpeb[:, 512*jc:512*(jc+1)],
                            op0=bypass, op1=add)
                        nc.sync.dma_start(
                            out=y_d[128*ti:128*(ti+1), 512*jc:512*(jc+1)],
                            in_=ysb[:])
        wpp.release()
        wkvp.release()
        per.release()

    nc.compile()
    return nc


def _get_nc():
    if "nc" not in _cache:
        _cache["nc"] = _build()
    return _cache["nc"]


def _host_prep(x, Wqkv, bqkv, Wproj, bproj):
    x = np.ascontiguousarray(np.asarray(x, dtype=np.float32))
    Wqkv = np.asarray(Wqkv, dtype=np.float32)
    bqkv = np.asarray(bqkv, dtype=np.float32)
    Wproj = np.ascontiguousarray(np.asarray(Wproj, dtype=np.float32))
    bproj = np.asarray(bproj, dtype=np.float32)

    wq = np.ascontiguousarray(Wqkv[:, :C] * np.float32(0.125)).astype(NPBF)
    wk = np.ascontiguousarray(Wqkv[:, C:2*C]).astype(NPBF)
    wv = np.ascontiguousarray(Wqkv[:, 2*C:]).astype(NPBF)
    wp = Wproj.astype(NPBF)
    bq8 = (bqkv[:C] * np.float32(0.125)).reshape(8, 128, 1).copy()
    bk8 = bqkv[C:2*C].reshape(8, 128, 1).copy()
    bv = bqkv[2*C:]
    bpe = (bproj.astype(np.float64) + bv.astype(np.float64) @ Wproj.astype(np.float64)).astype(np.float32)
    bpeb = np.ascontiguousarray(np.broadcast_to(bpe, (128, C)))

    ridx = np.arange(128)[:, None]
    cidx = np.arange(128)[None, :]
    tri = (ridx <= cidx)
    masks = []
    for par in range(2):
        mk = np.zeros((128, 16 * 128), dtype=NPBF)
        for m in range(16):
            g = OWNT[par][m // 2]
            if m < g:
                mk[:, 128*m:128*(m+1)] = 1
            elif m == g:
                mk[:, 128*m:128*(m+1)] = tri
        masks.append(mk)

    in_maps = []
    for core in range(8):
        b, par = core // 2, core % 2
        xt = np.ascontiguousarray(x[b].T.astype(NPBF))
        xq = np.ascontiguousarray(
            np.concatenate([xt[:, 128*g:128*(g+1)] for g in OWNT[par]], axis=1))
        in_maps.append(dict(xt=xt, xq=xq, wq=wq, wk=wk, wv=wv, wp=wp,
                            bq=bq8, bk=bk8, bpeb=bpeb, masks=masks[par]))
    return in_maps


def kernel(x, Wqkv, bqkv, Wproj, bproj):
    nc = _get_nc()
    in_maps = _host_prep(x, Wqkv, bqkv, Wproj, bproj)
    trace = bool(os.environ.get("BASS_TRACE"))
    res = run_bass_kernel_spmd(nc, in_maps, list(range(8)), trace=trace)
    _cache["last_exec_time_ns"] = res.exec_time_ns
    _cache["last_res"] = res
    out = np.empty((B, T, C), dtype=np.float32)
    for core in range(8):
        b, par = core // 2, core % 2
        y = res.results[core]["y"]
        for si, g in enumerate(OWNT[par]):
            out[b, 128*g:128*(g+1)] = y[128*si:128*(si+1)]
    return out
